# revision 1
# baseline (speedup 1.0000x reference)
"""Gated linear attention on 8 TRN2 NeuronCores.

Sharding: data-parallel over tokens. Core c handles tokens
[c*2048, (c+1)*2048) of the flattened (B*N, C) = (16384, 1024) sequence,
i.e. batch b = c//2, sequence half = c%2. The linear-attention kv state
(and k-sum) needs a reduction over each batch's full sequence, so cores
{2b, 2b+1} all-reduce a small (128, 520) fp32 buffer (kv state + k-sum
for 16 heads) and everything else is local.

Device layouts (per core):
  xt   [C, T]    bf16  x^T shard (host pre-transposes + casts)
  wq   [C, C]    bf16  Wqkv[:, :C]        (lhsT for q-proj, out [d, tok])
  wkv  [C, 2C]   bf16  Wqkv[:, C:3C]      (rhs for k/v-proj, out [tok, feat])
  wgt  [C, C]    bf16  Wg                 (lhsT for gate-proj, out [g, tok])
  wp   [C, C]    bf16  Wp                 (lhsT for out-proj, out [o, tok])
  y    [C, T]    bf16  output^T (host transposes back + casts fp32)

q and gates are computed feature-major ([feat, tok]); k and v token-major
([tok, feat]) so the kv einsum can contract over tokens on the partition
axis. Gates are transposed on the DMA xbar (bf16 128x128 tiles) for the
k side. elu(x)+1 is computed as min(exp(x),1) + max(x,0).
"""

import numpy as np
import ml_dtypes

import concourse.bass as bass
import concourse.bacc as bacc
import concourse.tile as tile
import concourse.mybir as mybir
from concourse.bass_utils import run_bass_kernel_spmd

F32 = mybir.dt.float32
BF16 = mybir.dt.bfloat16
AF = mybir.ActivationFunctionType
ALU = mybir.AluOpType

B, N, C = 4, 4096, 1024
H, D = 16, 64
NCORES = 8
T = B * N // NCORES          # 2048 tokens per core
KC = C // 128                # 8 contraction chunks
TB = 512                     # token tile (free dim)
NT = T // TB                 # 4 token tiles
NS = T // 128                # 16 token subchunks (partition-dim tiles)
C2 = 2 * C

REPLICA_GROUPS = [[0, 1], [2, 3], [4, 5], [6, 7]]


def build_nc():
    nc = bacc.Bacc(
        "TRN2", target_bir_lowering=False, debug=False, num_devices=NCORES
    )
    xt = nc.dram_tensor("xt", [C, T], BF16, kind="ExternalInput")
    wq = nc.dram_tensor("wq", [C, C], BF16, kind="ExternalInput")
    wkv = nc.dram_tensor("wkv", [C, C2], BF16, kind="ExternalInput")
    wgt = nc.dram_tensor("wgt", [C, C], BF16, kind="ExternalInput")
    wp = nc.dram_tensor("wp", [C, C], BF16, kind="ExternalInput")
    bg2 = nc.dram_tensor("bg2", [128, KC], F32, kind="ExternalInput")
    bp2 = nc.dram_tensor("bp2", [128, KC], F32, kind="ExternalInput")
    e_all = nc.dram_tensor("e_all", [H, C], BF16, kind="ExternalInput")
    y = nc.dram_tensor("y", [C, T], BF16, kind="ExternalOutput")

    with tile.TileContext(nc) as tc:
        build_body(nc, tc, xt, wq, wkv, wgt, wp, bg2, bp2, e_all, y)

    nc.compile()
    return nc


def build_body(nc, tc, xt, wq, wkv, wgt, wp, bg2, bp2, e_all, y):
    from contextlib import ExitStack

    with ExitStack() as st:
        constp = st.enter_context(tc.tile_pool(name="constp", bufs=1))
        wbig = st.enter_context(tc.tile_pool(name="wbig", bufs=1))
        wsmall = st.enter_context(tc.tile_pool(name="wsmall", bufs=1))
        big1 = st.enter_context(tc.tile_pool(name="big1", bufs=1))
        gatesp = st.enter_context(tc.tile_pool(name="gatesp", bufs=1))
        qp = st.enter_context(tc.tile_pool(name="qp", bufs=1))
        workp = st.enter_context(tc.tile_pool(name="workp", bufs=2))
        elup = st.enter_context(tc.tile_pool(name="elup", bufs=2))
        mmps = st.enter_context(tc.tile_pool(name="mmps", bufs=6, space="PSUM"))
        einps = st.enter_context(tc.tile_pool(name="einps", bufs=2, space="PSUM"))
        dramp = st.enter_context(tc.tile_pool(name="dramp", bufs=1, space="DRAM"))

        # ------------------------------------------------ constants / weights
        bg_sb = constp.tile([128, KC], F32, name="bg_sb")
        nc.sync.dma_start(bg_sb[:], bg2[:])
        bp_sb = constp.tile([128, KC], F32, name="bp_sb")
        nc.sync.dma_start(bp_sb[:], bp2[:])
        e_sb = constp.tile([H, C], BF16, name="e_sb")
        nc.sync.dma_start(e_sb[:], e_all[:])

        # chunked loads in consumption order so the first matmuls start early
        wkv_sb = wbig.tile([128, KC * C2], BF16, name="wkv_sb", tag="wbig")
        wg_sb = wsmall.tile([128, KC * C], BF16, name="wg_sb", tag="wsmall")
        xt_sb = big1.tile([128, KC * T], BF16, name="xt_sb", tag="big1")
        for k in range(KC):
            nc.sync.dma_start(
                wg_sb[:, k * C : (k + 1) * C], wgt[k * 128 : (k + 1) * 128, :]
            )
            nc.sync.dma_start(
                xt_sb[:, k * T : (k + 1) * T], xt[k * 128 : (k + 1) * 128, :]
            )
        for k in range(KC):
            nc.sync.dma_start(
                wkv_sb[:, k * C2 : (k + 1) * C2], wkv[k * 128 : (k + 1) * 128, :]
            )

        # ------------------------------------------------ phase 1: gates
        # gates[g, tok] = sigmoid(x @ Wg + bg)^T, feature-major
        gates_sb = gatesp.tile([128, KC * T], BF16, name="gates_sb", tag="gates")
        for m in range(KC):
            gps = [
                mmps.tile([128, TB], F32, name=f"gps{n}", tag="mm")
                for n in range(NT)
            ]
            for k in range(KC):
                lhsT = wg_sb[:, k * C + m * 128 : k * C + (m + 1) * 128]
                for n in range(NT):
                    nc.tensor.matmul(
                        gps[n][:],
                        lhsT=lhsT,
                        rhs=xt_sb[:, k * T + n * TB : k * T + (n + 1) * TB],
                        start=(k == 0),
                        stop=(k == KC - 1),
                    )
            for n in range(NT):
                nc.scalar.activation(
                    gates_sb[:, m * T + n * TB : m * T + (n + 1) * TB],
                    gps[n][:],
                    AF.Sigmoid,
                    bias=bg_sb[:, m : m + 1],
                    scale=1.0,
                )

        # gates transpose, one big xbar op per gate chunk:
        # gT_full[p, m*T + s*128 + c] = gates[m*128 + c, s*128 + p]
        gT_full = qp.tile([128, KC * T], BF16, name="gT_full", tag="gT_full")
        for m in range(KC):
            nc.sync.dma_start(
                gT_full[:, m * T : (m + 1) * T].rearrange(
                    "p (s c) -> p s c", c=128
                ),
                gates_sb[:, m * T : (m + 1) * T],
                transpose=True,
            )
        gT4 = gT_full.rearrange("p (m s c) -> p m s c", s=NS, c=128)

        # ------------------------------------------------ phase 1: k/v + kv state
        # kv einsum is PSUM-accumulated over 512-token groups (a PSUM "zero
        # region" admits only one open accumulation group per bank x
        # partition-range), then DVE-accumulated into SBUF.
        # kv_acc block p = cols [130p, 130p+130):
        #   rows 0:64,  cols +0:65   = kv_aug head 2p   (col 64 = k_sum)
        #   rows 64:128, cols +65:130 = kv_aug head 2p+1 (col 129 = k_sum)
        kv_acc = constp.tile([128, KC * 130], F32, name="kv_acc")
        for g in range(NT):
            kbfs, vaugs = [], []
            for si in range(4):
                s = g * 4 + si
                # k/v projection, token-major: out [tok, 2048]
                kvps = [
                    mmps.tile([128, TB], F32, name=f"kvps{n}", tag="mm")
                    for n in range(4)
                ]
                for k in range(KC):
                    lhsT = xt_sb[:, k * T + s * 128 : k * T + (s + 1) * 128]
                    for n in range(4):
                        nc.tensor.matmul(
                            kvps[n][:],
                            lhsT=lhsT,
                            rhs=wkv_sb[:, k * C2 + n * TB : k * C2 + (n + 1) * TB],
                            start=(k == 0),
                            stop=(k == KC - 1),
                        )
                # k = elu(k_raw * g) + 1 = min(exp(kg),1) + max(kg,0)
                k_bf = workp.tile([128, C], BF16, name="k_bf", tag="k_bf", bufs=5)
                for n in range(2):
                    kg = elup.tile([128, TB], F32, name="kg", tag="kg")
                    nc.vector.tensor_mul(
                        kg.rearrange("p (m c) -> p m c", c=128),
                        kvps[n].rearrange("p (m c) -> p m c", c=128),
                        gT4[:, 4 * n : 4 * n + 4, s, :],
                    )
                    relu = elup.tile([128, TB], BF16, name="relu", tag="relu")
                    nc.vector.tensor_scalar_max(relu[:], kg[:], 0.0)
                    ex = elup.tile([128, TB], BF16, name="ex", tag="ex")
                    nc.scalar.activation(ex[:], kg[:], AF.Exp)
                    nc.vector.scalar_tensor_tensor(
                        k_bf[:, n * TB : (n + 1) * TB],
                        in0=ex[:],
                        scalar=1.0,
                        in1=relu[:],
                        op0=ALU.min,
                        op1=ALU.add,
                    )
                # v, augmented with ones column per head (yields k_sum)
                v_aug = workp.tile(
                    [128, H * 65], BF16, name="v_aug", tag="v_aug", bufs=5
                )
                v3 = v_aug.rearrange("p (h e) -> p h e", e=65)
                nc.vector.memset(v3[:, :, 64:65], 1.0)
                for n in range(2, 4):
                    h0 = (n - 2) * 8
                    nc.scalar.copy(
                        v3[:, h0 : h0 + 8, 0:64],
                        kvps[n].rearrange("p (h e) -> p h e", e=64),
                    )
                kbfs.append(k_bf)
                vaugs.append(v_aug)
            # kv einsum for this 512-token group, head pairs packed [128, 130]
            for p in range(KC):
                eps = einps.tile([128, 130], F32, name="eps", tag="ein")
                for si in range(4):
                    nc.tensor.matmul(
                        eps[:],
                        lhsT=kbfs[si][:, 128 * p : 128 * (p + 1)],
                        rhs=vaugs[si][:, 130 * p : 130 * (p + 1)],
                        start=(si == 0),
                        stop=(si == 3),
                    )
                if g == 0:
                    nc.vector.tensor_copy(
                        kv_acc[:, 130 * p : 130 * (p + 1)], eps[:]
                    )
                else:
                    nc.vector.tensor_add(
                        kv_acc[:, 130 * p : 130 * (p + 1)],
                        kv_acc[:, 130 * p : 130 * (p + 1)],
                        eps[:],
                    )

        # q weights load, emitted before the AR bounce DMA so the in-order
        # sync queue starts it as soon as the wg slot frees (~mid phase 1)
        wq_sb = wsmall.tile([128, KC * C], BF16, name="wq_sb", tag="wsmall")
        nc.sync.dma_start(wq_sb.rearrange("p (k n) -> p k n", k=KC), wq.rearrange("(k p) n -> p k n", p=128))

        # ------------------------------------------------ kv all-reduce (pairs)
        # compact to [128, 8*65]: head 2p at [0:64, 65p:65p+65],
        # head 2p+1 at [64:128, 65p:65p+65]
        kv_cat = constp.tile([128, KC * 65], F32, name="kv_cat", tag="kv_cat")
        nc.vector.tensor_copy(
            kv_cat[0:64, :].rearrange("p (j e) -> p j e", e=65),
            kv_acc[0:64, :].rearrange("p (j q) -> p j q", q=130)[:, :, 0:65],
        )
        nc.vector.tensor_copy(
            kv_cat[64:128, :].rearrange("p (j e) -> p j e", e=65),
            kv_acc[64:128, :].rearrange("p (j q) -> p j q", q=130)[:, :, 65:130],
        )
        bounce_in = dramp.tile([128, KC * 65], F32, name="bounce_in")
        bounce_out = dramp.tile([128, KC * 65], F32, name="bounce_out")
        nc.sync.dma_start(bounce_in[:], kv_cat[:])
        nc.gpsimd.collective_compute(
            "AllReduce",
            ALU.add,
            replica_groups=REPLICA_GROUPS,
            ins=[bounce_in.opt()],
            outs=[bounce_out.opt()],
        )
        kv_f32 = constp.tile([128, KC * 65], F32, name="kv_f32", tag="kv_cat")
        nc.sync.dma_start(kv_f32[:], bounce_out[:])
        kv_bf = constp.tile([128, KC * 65], BF16, name="kv_bf")
        nc.vector.tensor_copy(kv_bf[:], kv_f32[:])
        # block-diagonal kv tiles for the attention matmul (K=128 per pair)
        bds = []
        for j in range(KC):
            bd = constp.tile([128, 128], BF16, name=f"bd{j}")
            nc.vector.memset(bd[:], 0.0)
            nc.vector.tensor_copy(bd[0:64, 0:64], kv_bf[0:64, 65 * j : 65 * j + 64])
            nc.vector.tensor_copy(
                bd[64:128, 64:128], kv_bf[64:128, 65 * j : 65 * j + 64]
            )
            bds.append(bd)
        # block-diagonal k_sum tiles for the normalizer matmul
        blks = []
        for j in range(KC):
            bj = constp.tile([128, H], BF16, name=f"blk{j}")
            nc.vector.memset(bj[:], 0.0)
            for par in range(2):
                h = 2 * j + par
                nc.vector.tensor_copy(
                    bj[par * 64 : (par + 1) * 64, h : h + 1],
                    kv_bf[par * 64 : (par + 1) * 64, 65 * j + 64 : 65 * j + 65],
                )
            blks.append(bj)

        # ------------------------------------------------ phase 1.5: q (overlaps AR)
        q_sb = wbig.tile([128, KC * T], BF16, name="q_sb", tag="wbig")
        for m in range(KC):
            qps = [
                mmps.tile([128, TB], F32, name=f"qps{n}", tag="mm")
                for n in range(NT)
            ]
            for k in range(KC):
                lhsT = wq_sb[:, k * C + m * 128 : k * C + (m + 1) * 128]
                for n in range(NT):
                    nc.tensor.matmul(
                        qps[n][:],
                        lhsT=lhsT,
                        rhs=xt_sb[:, k * T + n * TB : k * T + (n + 1) * TB],
                        start=(k == 0),
                        stop=(k == KC - 1),
                    )
            for n in range(NT):
                qg = elup.tile([128, TB], F32, name="qg", tag="kg")
                nc.vector.tensor_mul(
                    qg[:],
                    qps[n][:],
                    gates_sb[:, m * T + n * TB : m * T + (n + 1) * TB],
                )
                relu = elup.tile([128, TB], BF16, name="relu2", tag="relu")
                nc.vector.tensor_scalar_max(relu[:], qg[:], 0.0)
                ex = elup.tile([128, TB], BF16, name="ex2", tag="ex")
                nc.scalar.activation(ex[:], qg[:], AF.Exp)
                nc.vector.scalar_tensor_tensor(
                    q_sb[:, m * T + n * TB : m * T + (n + 1) * TB],
                    in0=ex[:],
                    scalar=1.0,
                    in1=relu[:],
                    op0=ALU.min,
                    op1=ALU.add,
                )

        # ------------------------------------------------ phase 2: attention + proj
        wp_sb = wsmall.tile([128, KC * C], BF16, name="wp_sb", tag="wsmall")
        nc.sync.dma_start(wp_sb.rearrange("p (k n) -> p k n", k=KC), wp.rearrange("(k p) n -> p k n", p=128))

        # normalizer reciprocals for all token chunks up front (keeps the
        # recip chain off the per-chunk critical path)
        rbs = []
        for n in range(NT):
            # norm[h, tok] = sum_d q[h*64+d, tok] * k_sum[h, d]
            nps = mmps.tile([H, TB], F32, name="nps", tag="mm")
            for j in range(KC):
                nc.tensor.matmul(
                    nps[:],
                    lhsT=blks[j][:],
                    rhs=q_sb[:, j * T + n * TB : j * T + (n + 1) * TB],
                    start=(j == 0),
                    stop=(j == KC - 1),
                )
            nc.vector.tensor_scalar_add(nps[:], nps[:], 1e-8)
            nrec = elup.tile([H, TB], F32, name="nrec", tag="nrec")
            nc.vector.reciprocal_approx_fast(nrec[:], nps[:])
            rb = constp.tile([H, TB], BF16, name=f"rb{n}")
            nc.vector.tensor_copy(rb[:], nrec[:])
            rbs.append(rb)

        # attention + projection, streamed per token chunk
        for n in range(NT):
            # attn[e, tok] = (q @ kv) * bcast(recip)   (feature-major)
            attn_n = big1.tile([128, KC * TB], BF16, name="attn_n", tag="big1")
            for j in range(KC):
                pps = mmps.tile([128, TB], F32, name="pps", tag="mm")
                nc.tensor.matmul(
                    pps[:],
                    lhsT=bds[j][:],
                    rhs=q_sb[:, j * T + n * TB : j * T + (n + 1) * TB],
                    start=True,
                    stop=True,
                )
                bps = mmps.tile([128, TB], F32, name="bps", tag="mm")
                nc.tensor.matmul(
                    bps[:],
                    lhsT=e_sb[:, j * 128 : (j + 1) * 128],
                    rhs=rbs[n][:],
                    start=True,
                    stop=True,
                )
                # DVE can read only one PSUM operand per op: stage the
                # broadcast through SBUF on the scalar engine first.
                bc_sb = elup.tile([128, TB], BF16, name="bc_sb", tag="bc_sb")
                nc.scalar.copy(bc_sb[:], bps[:])
                nc.vector.tensor_mul(
                    attn_n[:, j * TB : (j + 1) * TB],
                    pps[:],
                    bc_sb[:],
                )

            # output projection for this chunk: y[o, tok] = Wp^T @ attn + bp
            for m in range(KC):
                ops_ = mmps.tile([128, TB], F32, name="ops", tag="mm")
                for j in range(KC):
                    nc.tensor.matmul(
                        ops_[:],
                        lhsT=wp_sb[:, j * C + m * 128 : j * C + (m + 1) * 128],
                        rhs=attn_n[:, j * TB : (j + 1) * TB],
                        start=(j == 0),
                        stop=(j == KC - 1),
                    )
                o = gatesp.tile([128, TB], BF16, name="o", tag="gates")
                nc.scalar.activation(
                    o[:],
                    ops_[:],
                    AF.Identity,
                    bias=bp_sb[:, m : m + 1],
                    scale=1.0,
                )
                nc.sync.dma_start(
                    y[128 * m : 128 * (m + 1), n * TB : (n + 1) * TB], o[:]
                )


_NC_CACHE = {}


def get_nc():
    if "nc" not in _NC_CACHE:
        _NC_CACHE["nc"] = build_nc()
    return _NC_CACHE["nc"]


def make_in_maps(x, Wqkv, Wg, bg, Wp, bp):
    bf = ml_dtypes.bfloat16
    x = np.asarray(x, dtype=np.float32)
    Wqkv = np.asarray(Wqkv, dtype=np.float32)
    Wg = np.asarray(Wg, dtype=np.float32)
    bg = np.asarray(bg, dtype=np.float32)
    Wp = np.asarray(Wp, dtype=np.float32)
    bp = np.asarray(bp, dtype=np.float32)

    wq = np.ascontiguousarray(Wqkv[:, :C]).astype(bf)
    wkv = np.ascontiguousarray(Wqkv[:, C:]).astype(bf)
    wgt = Wg.astype(bf)
    wp = Wp.astype(bf)
    bg2 = np.ascontiguousarray(bg.reshape(KC, 128).T)
    bp2 = np.ascontiguousarray(bp.reshape(KC, 128).T)
    e_all = np.zeros((H, C), dtype=bf)
    for h in range(H):
        e_all[h, h * D : (h + 1) * D] = 1.0

    xf = x.reshape(NCORES, T, C)
    in_maps = []
    for c in range(NCORES):
        xtc = np.ascontiguousarray(xf[c].T).astype(bf)
        in_maps.append(
            dict(
                xt=xtc, wq=wq, wkv=wkv, wgt=wgt, wp=wp,
                bg2=bg2, bp2=bp2, e_all=e_all,
            )
        )
    return in_maps


def kernel(x, Wqkv, Wg, bg, Wp, bp, _collect_perf=None):
    nc = get_nc()
    in_maps = make_in_maps(x, Wqkv, Wg, bg, Wp, bp)
    kwargs = {}
    if _collect_perf is not None:
        kwargs = dict(trace=True)
        if _collect_perf.get("tmpdir"):
            kwargs["tmpdir"] = _collect_perf["tmpdir"]
    res = run_bass_kernel_spmd(
        nc, in_maps, core_ids=list(range(NCORES)), **kwargs
    )
    if _collect_perf is not None:
        _collect_perf["exec_time_ns"] = res.exec_time_ns
        _collect_perf["results"] = res
    out = np.empty((NCORES, T, C), dtype=np.float32)
    for c in range(NCORES):
        out[c] = res.results[c]["y"].astype(np.float32).T
    return out.reshape(B, N, C)



# revision 2
# speedup vs baseline: 1.0163x; 1.0163x over previous
"""Gated linear attention on 8 TRN2 NeuronCores.

Sharding: data-parallel over tokens. Core c handles tokens
[c*2048, (c+1)*2048) of the flattened (B*N, C) = (16384, 1024) sequence,
i.e. batch b = c//2, sequence half = c%2. The linear-attention kv state
(and k-sum) needs a reduction over each batch's full sequence, so cores
{2b, 2b+1} all-reduce a small (128, 520) fp32 buffer (kv state + k-sum
for 16 heads) and everything else is local.

Pipeline layout (tuned for continuous PE occupancy):
  phase 1, streamed per 512-token slab g: gate-proj -> sigmoid ->
    xbar transpose; k/v-proj -> elu(k*g)+1 -> kv einsum (PSUM-accumulated
    per slab, DVE-accumulated into SBUF).
  AR launch (bounce DRAM + pair all-reduce on gpsimd queue).
  phase 1.5/2 interleaved per token block n: q-proj + elu (AR hides under
    the first two blocks), then attention (q@kv, normalizer via a
    ksum-broadcast matmul, reciprocal+mul on DVE) and the output
    projection, emitted in an order that keeps PE dense:
    q0 q1 [AR unpack] pn0 q2 pn1 op0 q3 pn2 op1 pn3 op2 op3.

Engine queues: weights load on gpsimd (cheap issue), x slabs / transposes
/ y stores on sync, consts on scalar. The normalizer uses a per-head-pair
lhsT with ksum replicated across 64 columns so the matmul broadcasts
norm[h, tok] directly to all 64 feature rows of head h (no separate
broadcast matmul + PSUM-staging copy).
"""

import numpy as np
import ml_dtypes

import concourse.bass as bass
import concourse.bacc as bacc
import concourse.tile as tile
import concourse.mybir as mybir
from concourse.bass_utils import run_bass_kernel_spmd

F32 = mybir.dt.float32
BF16 = mybir.dt.bfloat16
AF = mybir.ActivationFunctionType
ALU = mybir.AluOpType

B, N, C = 4, 4096, 1024
H, D = 16, 64
NCORES = 8
T = B * N // NCORES          # 2048 tokens per core
KC = C // 128                # 8 contraction chunks
TB = 512                     # token tile (free dim)
NT = T // TB                 # 4 token tiles / slabs
NS = T // 128                # 16 token subchunks (partition-dim tiles)
C2 = 2 * C

REPLICA_GROUPS = [[0, 1], [2, 3], [4, 5], [6, 7]]


def build_nc():
    nc = bacc.Bacc(
        "TRN2", target_bir_lowering=False, debug=False, num_devices=NCORES
    )
    xt = nc.dram_tensor("xt", [C, T], BF16, kind="ExternalInput")
    wq = nc.dram_tensor("wq", [C, C], BF16, kind="ExternalInput")
    wkv = nc.dram_tensor("wkv", [C, C2], BF16, kind="ExternalInput")
    wgt = nc.dram_tensor("wgt", [C, C], BF16, kind="ExternalInput")
    wp = nc.dram_tensor("wp", [C, C], BF16, kind="ExternalInput")
    bg2 = nc.dram_tensor("bg2", [128, KC], F32, kind="ExternalInput")
    bp2 = nc.dram_tensor("bp2", [128, KC], F32, kind="ExternalInput")
    y = nc.dram_tensor("y", [C, T], BF16, kind="ExternalOutput")

    with tile.TileContext(nc) as tc:
        build_body(nc, tc, xt, wq, wkv, wgt, wp, bg2, bp2, y)

    nc.compile()
    return nc


def build_body(nc, tc, xt, wq, wkv, wgt, wp, bg2, bp2, y):
    from contextlib import ExitStack

    with ExitStack() as st:
        constp = st.enter_context(tc.tile_pool(name="constp", bufs=1))
        wsmall = st.enter_context(tc.tile_pool(name="wsmall", bufs=1))
        wbig = st.enter_context(tc.tile_pool(name="wbig", bufs=1))
        xsp = st.enter_context(tc.tile_pool(name="xsp", bufs=4))
        gatesp = st.enter_context(tc.tile_pool(name="gatesp", bufs=1))
        gtp = st.enter_context(tc.tile_pool(name="gtp", bufs=1))
        workp = st.enter_context(tc.tile_pool(name="workp", bufs=4))
        elup = st.enter_context(tc.tile_pool(name="elup", bufs=2))
        outp = st.enter_context(tc.tile_pool(name="outp", bufs=3))
        mmps = st.enter_context(tc.tile_pool(name="mmps", bufs=5, space="PSUM"))
        pnps = st.enter_context(tc.tile_pool(name="pnps", bufs=3, space="PSUM"))
        dramp = st.enter_context(tc.tile_pool(name="dramp", bufs=1, space="DRAM"))

        # ------------------------------------------------ loads
        # weights on the gpsimd queue (25ns/issue), x slabs on sync: the two
        # streams issue concurrently so the first gate matmul starts ~2.5us in.
        bg_sb = constp.tile([128, KC], F32, name="bg_sb")
        nc.scalar.dma_start(bg_sb[:], bg2[:])
        bp_sb = constp.tile([128, KC], F32, name="bp_sb")
        nc.scalar.dma_start(bp_sb[:], bp2[:])

        wg_sb = wsmall.tile([128, KC * C], BF16, name="wg_sb", tag="ws", bufs=1)
        for k in range(KC):
            nc.gpsimd.dma_start(
                wg_sb[:, k * C : (k + 1) * C], wgt[k * 128 : (k + 1) * 128, :]
            )
        wkv_sb = wbig.tile([128, KC * C2], BF16, name="wkv_sb", tag="wb", bufs=1)
        for k in range(KC):
            nc.gpsimd.dma_start(
                wkv_sb[:, k * C2 : (k + 1) * C2], wkv[k * 128 : (k + 1) * 128, :]
            )
        xs = []
        for g in range(NT):
            xg = xsp.tile([128, KC * TB], BF16, name=f"xs{g}", tag="xs", bufs=4)
            for k in range(KC):
                nc.sync.dma_start(
                    xg[:, k * TB : (k + 1) * TB],
                    xt[k * 128 : (k + 1) * 128, g * TB : (g + 1) * TB],
                )
            xs.append(xg)

        gates_sb = gatesp.tile([128, KC * T], BF16, name="gates_sb", tag="g")
        gT_full = gtp.tile([128, KC * T], BF16, name="gT_full", tag="gt", bufs=1)
        gT4 = gT_full.rearrange("p (m s c) -> p m s c", s=NS, c=128)
        kv_acc = constp.tile([128, KC * 130], F32, name="kv_acc")

        # ------------------------------------------------ phase 1 (per slab)
        for g in range(NT):
            # gates[m*128+c, tok] = sigmoid(x @ Wg + bg)^T, feature-major
            for mh in range(2):
                gps = [
                    mmps.tile([128, TB], F32, name=f"gps{mi}", tag="mm")
                    for mi in range(4)
                ]
                for k in range(KC):
                    for mi in range(4):
                        m = mh * 4 + mi
                        nc.tensor.matmul(
                            gps[mi][:],
                            lhsT=wg_sb[:, k * C + m * 128 : k * C + (m + 1) * 128],
                            rhs=xs[g][:, k * TB : (k + 1) * TB],
                            start=(k == 0),
                            stop=(k == KC - 1),
                        )
                for mi in range(4):
                    m = mh * 4 + mi
                    nc.scalar.activation(
                        gates_sb[:, m * T + g * TB : m * T + (g + 1) * TB],
                        gps[mi][:],
                        AF.Sigmoid,
                        bias=bg_sb[:, m : m + 1],
                        scale=1.0,
                    )
            # per-slab transpose on the DMA xbar (token-major gates for k)
            for m in range(KC):
                nc.sync.dma_start(
                    gT_full[
                        :, m * T + g * TB : m * T + (g + 1) * TB
                    ].rearrange("p (s c) -> p s c", c=128),
                    gates_sb[:, m * T + g * TB : m * T + (g + 1) * TB],
                    transpose=True,
                )

            # k/v projection + elu + kv einsum for the 4 si of this slab
            kbfs, vaugs = [], []
            for si in range(4):
                s = g * 4 + si
                kvps = [
                    mmps.tile([128, TB], F32, name=f"kvps{nn}", tag="mm")
                    for nn in range(4)
                ]
                for k in range(KC):
                    lhsT = xs[g][:, k * TB + si * 128 : k * TB + (si + 1) * 128]
                    for nn in range(4):
                        nc.tensor.matmul(
                            kvps[nn][:],
                            lhsT=lhsT,
                            rhs=wkv_sb[:, k * C2 + nn * TB : k * C2 + (nn + 1) * TB],
                            start=(k == 0),
                            stop=(k == KC - 1),
                        )
                # k = elu(k_raw * g) + 1 = min(exp(kg),1) + max(kg,0)
                # (relu on the scalar engine: same act table as exp)
                k_bf = workp.tile([128, C], BF16, name="k_bf", tag="k_bf", bufs=4)
                for nn in range(2):
                    kg = elup.tile([128, TB], F32, name="kg", tag="kg")
                    nc.vector.tensor_mul(
                        kg.rearrange("p (m c) -> p m c", c=128),
                        kvps[nn].rearrange("p (m c) -> p m c", c=128),
                        gT4[:, 4 * nn : 4 * nn + 4, s, :],
                    )
                    relu = elup.tile([128, TB], BF16, name="relu", tag="relu")
                    nc.scalar.activation(relu[:], kg[:], AF.Relu)
                    ex = elup.tile([128, TB], BF16, name="ex", tag="ex")
                    nc.scalar.activation(ex[:], kg[:], AF.Exp)
                    nc.vector.scalar_tensor_tensor(
                        k_bf[:, nn * TB : (nn + 1) * TB],
                        in0=ex[:],
                        scalar=1.0,
                        in1=relu[:],
                        op0=ALU.min,
                        op1=ALU.add,
                    )
                # v, augmented with ones column per head (yields k_sum)
                v_aug = workp.tile(
                    [128, H * 65], BF16, name="v_aug", tag="v_aug", bufs=4
                )
                v3 = v_aug.rearrange("p (h e) -> p h e", e=65)
                nc.vector.memset(v3[:, :, 64:65], 1.0)
                for nn in range(2, 4):
                    h0 = (nn - 2) * 8
                    nc.scalar.copy(
                        v3[:, h0 : h0 + 8, 0:64],
                        kvps[nn].rearrange("p (h e) -> p h e", e=64),
                    )
                kbfs.append(k_bf)
                vaugs.append(v_aug)
            # kv einsum for this slab, head pairs packed [128, 130]:
            #   rows 0:64,  cols +0:65   = kv_aug head 2p   (col 64 = k_sum)
            #   rows 64:128, cols +65:130 = kv_aug head 2p+1 (col 129 = k_sum)
            for p in range(KC):
                eps = pnps.tile([128, TB], F32, name="eps", tag="pn")
                for si in range(4):
                    nc.tensor.matmul(
                        eps[:, 0:130],
                        lhsT=kbfs[si][:, 128 * p : 128 * (p + 1)],
                        rhs=vaugs[si][:, 130 * p : 130 * (p + 1)],
                        start=(si == 0),
                        stop=(si == 3),
                    )
                if g == 0:
                    nc.vector.tensor_copy(
                        kv_acc[:, 130 * p : 130 * (p + 1)], eps[:, 0:130]
                    )
                else:
                    nc.vector.tensor_add(
                        kv_acc[:, 130 * p : 130 * (p + 1)],
                        kv_acc[:, 130 * p : 130 * (p + 1)],
                        eps[:, 0:130],
                    )

        # ------------------------------------------------ kv all-reduce (pairs)
        # compact to [128, 8*65]: head 2p at [0:64, 65p:65p+65],
        # head 2p+1 at [64:128, 65p:65p+65]
        kv_cat = constp.tile([128, KC * 65], F32, name="kv_cat", tag="kvc")
        nc.vector.tensor_copy(
            kv_cat[0:64, :].rearrange("p (j e) -> p j e", e=65),
            kv_acc[0:64, :].rearrange("p (j q) -> p j q", q=130)[:, :, 0:65],
        )
        nc.vector.tensor_copy(
            kv_cat[64:128, :].rearrange("p (j e) -> p j e", e=65),
            kv_acc[64:128, :].rearrange("p (j q) -> p j q", q=130)[:, :, 65:130],
        )
        bounce_in = dramp.tile([128, KC * 65], F32, name="bounce_in")
        bounce_out = dramp.tile([128, KC * 65], F32, name="bounce_out")
        nc.sync.dma_start(bounce_in[:], kv_cat[:])
        # wp load waits for the gT slot (last elu read), well before phase 2
        wp_sb = gtp.tile([128, KC * C], BF16, name="wp_sb", tag="gt", bufs=1)
        nc.gpsimd.dma_start(
            wp_sb.rearrange("p (k n) -> p k n", k=KC),
            wp.rearrange("(k p) n -> p k n", p=128),
        )
        nc.gpsimd.collective_compute(
            "AllReduce",
            ALU.add,
            replica_groups=REPLICA_GROUPS,
            ins=[bounce_in.opt()],
            outs=[bounce_out.opt()],
        )

        wq_sb = wsmall.tile([128, KC * C], BF16, name="wq_sb", tag="ws", bufs=1)
        nc.gpsimd.dma_start(
            wq_sb.rearrange("p (k n) -> p k n", k=KC),
            wq.rearrange("(k p) n -> p k n", p=128),
        )
        q_sb = wbig.tile([128, KC * T], BF16, name="q_sb", tag="wb", bufs=1)

        # ---------------------------------------- phase 1.5/2 (interleaved)
        def q_block(n):
            # q[m*128+c, tok] = elu(q_raw * g) + 1 for token block n
            for mh in range(2):
                qps = [
                    mmps.tile([128, TB], F32, name=f"qps{mi}", tag="mm")
                    for mi in range(4)
                ]
                for k in range(KC):
                    for mi in range(4):
                        m = mh * 4 + mi
                        nc.tensor.matmul(
                            qps[mi][:],
                            lhsT=wq_sb[:, k * C + m * 128 : k * C + (m + 1) * 128],
                            rhs=xs[n][:, k * TB : (k + 1) * TB],
                            start=(k == 0),
                            stop=(k == KC - 1),
                        )
                for mi in range(4):
                    m = mh * 4 + mi
                    qg = elup.tile([128, TB], F32, name="qg", tag="kg")
                    nc.vector.tensor_mul(
                        qg[:],
                        qps[mi][:],
                        gates_sb[:, m * T + n * TB : m * T + (n + 1) * TB],
                    )
                    relu = elup.tile([128, TB], BF16, name="relu2", tag="relu")
                    nc.scalar.activation(relu[:], qg[:], AF.Relu)
                    ex = elup.tile([128, TB], BF16, name="ex2", tag="ex")
                    nc.scalar.activation(ex[:], qg[:], AF.Exp)
                    nc.vector.scalar_tensor_tensor(
                        q_sb[:, m * T + n * TB : m * T + (n + 1) * TB],
                        in0=ex[:],
                        scalar=1.0,
                        in1=relu[:],
                        op0=ALU.min,
                        op1=ALU.add,
                    )

        def unpack_ar():
            # recv + build the attention lhsTs; emitted mid-q so the queues
            # absorb it while the AR is already done
            kv_f32 = constp.tile([128, KC * 65], F32, name="kv_f32", tag="kvc")
            nc.sync.dma_start(kv_f32[:], bounce_out[:])
            kv_bf = constp.tile([128, KC * 65], BF16, name="kv_bf")
            nc.vector.tensor_copy(kv_bf[:], kv_f32[:])
            # block-diagonal kv (K=128 per head pair j)
            bds_all = constp.tile([128, KC * 128], BF16, name="bds_all")
            nc.vector.memset(bds_all[:], 0.0)
            nc.vector.tensor_copy(
                bds_all[0:64, :].rearrange("p (j c) -> p j c", c=128)[:, :, 0:64],
                kv_bf[0:64, :].rearrange("p (j q) -> p j q", q=65)[:, :, 0:64],
            )
            nc.vector.tensor_copy(
                bds_all[64:128, :].rearrange("p (j c) -> p j c", c=128)[
                    :, :, 64:128
                ],
                kv_bf[64:128, :].rearrange("p (j q) -> p j q", q=65)[:, :, 0:64],
            )
            # normalizer lhsT: ksum of head h replicated across that head's
            # 64 output columns, so norm broadcasts straight out of the PE
            nb_all = constp.tile([128, KC * 128], BF16, name="nb_all")
            nc.vector.memset(nb_all[:], 0.0)
            for j in range(KC):
                nc.scalar.activation(
                    nb_all[0:64, j * 128 : j * 128 + 64],
                    kv_f32[0:64, 0:64],
                    AF.Identity,
                    bias=kv_f32[0:64, 65 * j + 64 : 65 * j + 65],
                    scale=0.0,
                )
                nc.scalar.activation(
                    nb_all[64:128, j * 128 + 64 : j * 128 + 128],
                    kv_f32[64:128, 0:64],
                    AF.Identity,
                    bias=kv_f32[64:128, 65 * j + 64 : 65 * j + 65],
                    scale=0.0,
                )
            return bds_all, nb_all

        def pn_block(n, bds_all, nb_all):
            # attn[e, tok] = (q @ kv) / (q . k_sum), feature-major
            attn = xsp.tile([128, KC * TB], BF16, name=f"attn{n}", tag="xs", bufs=4)
            for j in range(KC):
                pps = pnps.tile([128, TB], F32, name="pps", tag="pn")
                nc.tensor.matmul(
                    pps[:],
                    lhsT=bds_all[:, 128 * j : 128 * (j + 1)],
                    rhs=q_sb[:, j * T + n * TB : j * T + (n + 1) * TB],
                    start=True,
                    stop=True,
                )
                nrm = pnps.tile([128, TB], F32, name="nrm", tag="pn")
                nc.tensor.matmul(
                    nrm[:],
                    lhsT=nb_all[:, 128 * j : 128 * (j + 1)],
                    rhs=q_sb[:, j * T + n * TB : j * T + (n + 1) * TB],
                    start=True,
                    stop=True,
                )
                rec = elup.tile([128, TB], F32, name="rec", tag="rec")
                nc.vector.reciprocal_approx_fast(rec[:], nrm[:])
                nc.vector.tensor_mul(
                    attn[:, j * TB : (j + 1) * TB], pps[:], rec[:]
                )
            return attn

        def op_block(n, attn):
            # y[o, tok] = Wp^T @ attn + bp for token block n
            for m in range(KC):
                ops_ = mmps.tile([128, TB], F32, name="ops", tag="mm")
                for j in range(KC):
                    nc.tensor.matmul(
                        ops_[:],
                        lhsT=wp_sb[:, j * C + m * 128 : j * C + (m + 1) * 128],
                        rhs=attn[:, j * TB : (j + 1) * TB],
                        start=(j == 0),
                        stop=(j == KC - 1),
                    )
                o = outp.tile([128, TB], BF16, name="o", tag="o", bufs=3)
                nc.scalar.activation(
                    o[:],
                    ops_[:],
                    AF.Identity,
                    bias=bp_sb[:, m : m + 1],
                    scale=1.0,
                )
                nc.sync.dma_start(
                    y[128 * m : 128 * (m + 1), n * TB : (n + 1) * TB], o[:]
                )

        q_block(0)
        q_block(1)
        bds_all, nb_all = unpack_ar()
        a0 = pn_block(0, bds_all, nb_all)
        q_block(2)
        a1 = pn_block(1, bds_all, nb_all)
        op_block(0, a0)
        q_block(3)
        a2 = pn_block(2, bds_all, nb_all)
        op_block(1, a1)
        a3 = pn_block(3, bds_all, nb_all)
        op_block(2, a2)
        op_block(3, a3)


_NC_CACHE = {}


def get_nc():
    if "nc" not in _NC_CACHE:
        _NC_CACHE["nc"] = build_nc()
    return _NC_CACHE["nc"]


def make_in_maps(x, Wqkv, Wg, bg, Wp, bp):
    bf = ml_dtypes.bfloat16
    x = np.asarray(x, dtype=np.float32)
    Wqkv = np.asarray(Wqkv, dtype=np.float32)
    Wg = np.asarray(Wg, dtype=np.float32)
    bg = np.asarray(bg, dtype=np.float32)
    Wp = np.asarray(Wp, dtype=np.float32)
    bp = np.asarray(bp, dtype=np.float32)

    wq = np.ascontiguousarray(Wqkv[:, :C]).astype(bf)
    wkv = np.ascontiguousarray(Wqkv[:, C:]).astype(bf)
    wgt = Wg.astype(bf)
    wp = Wp.astype(bf)
    bg2 = np.ascontiguousarray(bg.reshape(KC, 128).T)
    bp2 = np.ascontiguousarray(bp.reshape(KC, 128).T)

    xf = x.reshape(NCORES, T, C)
    in_maps = []
    for c in range(NCORES):
        xtc = np.ascontiguousarray(xf[c].T).astype(bf)
        in_maps.append(
            dict(
                xt=xtc, wq=wq, wkv=wkv, wgt=wgt, wp=wp,
                bg2=bg2, bp2=bp2,
            )
        )
    return in_maps


def kernel(x, Wqkv, Wg, bg, Wp, bp, _collect_perf=None):
    nc = get_nc()
    in_maps = make_in_maps(x, Wqkv, Wg, bg, Wp, bp)
    kwargs = {}
    if _collect_perf is not None:
        kwargs = dict(trace=True)
        if _collect_perf.get("tmpdir"):
            kwargs["tmpdir"] = _collect_perf["tmpdir"]
    res = run_bass_kernel_spmd(
        nc, in_maps, core_ids=list(range(NCORES)), **kwargs
    )
    if _collect_perf is not None:
        _collect_perf["exec_time_ns"] = res.exec_time_ns
        _collect_perf["results"] = res
    out = np.empty((NCORES, T, C), dtype=np.float32)
    for c in range(NCORES):
        out[c] = res.results[c]["y"].astype(np.float32).T
    return out.reshape(B, N, C)


# revision 3
# speedup vs baseline: 1.0952x; 1.0777x over previous
"""Gated linear attention on 8 TRN2 NeuronCores.

Sharding: data-parallel over tokens. Core c handles tokens
[c*2048, (c+1)*2048) of the flattened (B*N, C) = (16384, 1024) sequence,
i.e. batch b = c//2, sequence half = c%2. The linear-attention kv state
(and k-sum) needs a reduction over each batch's full sequence, so cores
{2b, 2b+1} all-reduce a small (128, 520) fp32 buffer (kv state + k-sum
for 16 heads) and everything else is local.

Schedule notes (tuned against perfetto traces):
  - ALL weight/x loads issue in the first few us (weights on the gpsimd
    queue at ~25ns/issue, x slabs too) and complete by ~40us, so the
    all-reduce window has zero competing DMA traffic (big transfers
    during the collective delayed its trigger + RDMA sends by ~60us).
  - phase 1 streams per 512-token slab: gate-proj and k-proj both
    feature-major per m-chunk (sigmoid -> gate-mul -> elu fused per
    chunk, no cross-slab transpose barrier), k transposed to token-major
    AFTER the elu on the DMA xbar (sync queue), v token-major, then the
    kv einsum (PSUM per slab, DVE-accumulated into SBUF).
  - the AR bounce launches at phase-1 end; q-proj for the first two
    token blocks hides it. Attention normalizer uses a ksum-replicated
    lhsT so the matmul broadcasts norm[h, tok] to all 64 feature rows of
    head h directly (no separate broadcast matmul / PSUM-staging copy);
    reciprocal+multiply on DVE. PE emission order interleaves q blocks,
    attention blocks, and out-projection blocks to keep PE dense:
    q0 q1 [AR unpack] pn0 q2 pn1 op0 q3 pn2 op1 pn3 op2 op3.
"""

import numpy as np
import ml_dtypes

import concourse.bass as bass
import concourse.bacc as bacc
import concourse.tile as tile
import concourse.mybir as mybir
from concourse.bass_utils import run_bass_kernel_spmd

F32 = mybir.dt.float32
BF16 = mybir.dt.bfloat16
AF = mybir.ActivationFunctionType
ALU = mybir.AluOpType

B, N, C = 4, 4096, 1024
H, D = 16, 64
NCORES = 8
T = B * N // NCORES          # 2048 tokens per core
KC = C // 128                # 8 contraction chunks
TB = 512                     # token tile (free dim)
NT = T // TB                 # 4 token tiles / slabs
NS = T // 128                # 16 token subchunks (partition-dim tiles)
C2 = 2 * C

REPLICA_GROUPS = [[0, 1], [2, 3], [4, 5], [6, 7]]


def build_nc():
    nc = bacc.Bacc(
        "TRN2", target_bir_lowering=False, debug=False, num_devices=NCORES
    )
    xt = nc.dram_tensor("xt", [C, T], BF16, kind="ExternalInput")
    wq = nc.dram_tensor("wq", [C, C], BF16, kind="ExternalInput")
    wkv = nc.dram_tensor("wkv", [C, C2], BF16, kind="ExternalInput")
    wgt = nc.dram_tensor("wgt", [C, C], BF16, kind="ExternalInput")
    wp = nc.dram_tensor("wp", [C, C], BF16, kind="ExternalInput")
    bg2 = nc.dram_tensor("bg2", [128, KC], F32, kind="ExternalInput")
    bp2 = nc.dram_tensor("bp2", [128, KC], F32, kind="ExternalInput")
    y = nc.dram_tensor("y", [C, T], BF16, kind="ExternalOutput")

    with tile.TileContext(nc) as tc:
        build_body(nc, tc, xt, wq, wkv, wgt, wp, bg2, bp2, y)

    nc.compile()
    return nc


def build_body(nc, tc, xt, wq, wkv, wgt, wp, bg2, bp2, y):
    from contextlib import ExitStack

    with ExitStack() as st:
        constp = st.enter_context(tc.tile_pool(name="constp", bufs=1))
        wgp = st.enter_context(tc.tile_pool(name="wgp", bufs=1))
        wqp = st.enter_context(tc.tile_pool(name="wqp", bufs=1))
        wpp = st.enter_context(tc.tile_pool(name="wpp", bufs=1))
        wbig = st.enter_context(tc.tile_pool(name="wbig", bufs=1))
        xsp = st.enter_context(tc.tile_pool(name="xsp", bufs=4))
        gatesp = st.enter_context(tc.tile_pool(name="gatesp", bufs=1))
        ktp = st.enter_context(tc.tile_pool(name="ktp", bufs=2))
        workp = st.enter_context(tc.tile_pool(name="workp", bufs=4))
        elup = st.enter_context(tc.tile_pool(name="elup", bufs=2))
        outp = st.enter_context(tc.tile_pool(name="outp", bufs=4))
        mmps = st.enter_context(tc.tile_pool(name="mmps", bufs=5, space="PSUM"))
        pnps = st.enter_context(tc.tile_pool(name="pnps", bufs=3, space="PSUM"))
        dramp = st.enter_context(tc.tile_pool(name="dramp", bufs=1, space="DRAM"))

        # ------------------------------------------------ loads (all upfront)
        bg_sb = constp.tile([128, KC], F32, name="bg_sb")
        nc.scalar.dma_start(bg_sb[:], bg2[:])
        bp_sb = constp.tile([128, KC], F32, name="bp_sb")
        nc.scalar.dma_start(bp_sb[:], bp2[:])

        wg_sb = wgp.tile([128, KC * C], BF16, name="wg_sb")
        for k in range(KC):
            nc.gpsimd.dma_start(
                wg_sb[:, k * C : (k + 1) * C], wgt[k * 128 : (k + 1) * 128, :]
            )
        xs = []
        for g in range(NT):
            xg = xsp.tile([128, KC * TB], BF16, name=f"xs{g}", tag="xs", bufs=4)
            xs.append(xg)
        wkv_sb = wbig.tile([128, KC * C2], BF16, name="wkv_sb", tag="wb", bufs=1)
        for k in range(KC):
            nc.gpsimd.dma_start(
                xs[0][:, k * TB : (k + 1) * TB],
                xt[k * 128 : (k + 1) * 128, 0:TB],
            )
        for k in range(KC):
            nc.gpsimd.dma_start(
                wkv_sb[:, k * C2 : (k + 1) * C2], wkv[k * 128 : (k + 1) * 128, :]
            )
        for g in range(1, NT):
            for k in range(KC):
                nc.gpsimd.dma_start(
                    xs[g][:, k * TB : (k + 1) * TB],
                    xt[k * 128 : (k + 1) * 128, g * TB : (g + 1) * TB],
                )
        wq_sb = wqp.tile([128, KC * C], BF16, name="wq_sb")
        nc.gpsimd.dma_start(
            wq_sb.rearrange("p (k n) -> p k n", k=KC),
            wq.rearrange("(k p) n -> p k n", p=128),
        )
        wp_sb = wpp.tile([128, KC * C], BF16, name="wp_sb")
        nc.gpsimd.dma_start(
            wp_sb.rearrange("p (k n) -> p k n", k=KC),
            wp.rearrange("(k p) n -> p k n", p=128),
        )

        gates_sb = gatesp.tile([128, KC * T], BF16, name="gates_sb", tag="g")
        kv_acc = constp.tile([128, KC * 130], F32, name="kv_acc")

        # ------------------------------------------------ phase 1 (per slab)
        for g in range(NT):
            # k^T for this slab, token-major: kT4[p, si, m, c] =
            # k[feat m*128+c, tok si*128+p]
            k_T = ktp.tile([128, 4 * KC * 128], BF16, name="k_T", tag="kt", bufs=2)
            kT4 = k_T.rearrange("p (s m c) -> p s m c", s=4, c=128)
            # gate + k projections, feature-major, fused per m-chunk
            for m in range(KC):
                gps = mmps.tile([128, TB], F32, name="gps", tag="mm")
                kps = mmps.tile([128, TB], F32, name="kps", tag="mm")
                for k in range(KC):
                    nc.tensor.matmul(
                        gps[:],
                        lhsT=wg_sb[:, k * C + m * 128 : k * C + (m + 1) * 128],
                        rhs=xs[g][:, k * TB : (k + 1) * TB],
                        start=(k == 0),
                        stop=(k == KC - 1),
                    )
                    nc.tensor.matmul(
                        kps[:],
                        lhsT=wkv_sb[
                            :, k * C2 + m * 128 : k * C2 + (m + 1) * 128
                        ],
                        rhs=xs[g][:, k * TB : (k + 1) * TB],
                        start=(k == 0),
                        stop=(k == KC - 1),
                    )
                gsl = gates_sb[:, m * T + g * TB : m * T + (g + 1) * TB]
                nc.scalar.activation(
                    gsl, gps[:], AF.Sigmoid, bias=bg_sb[:, m : m + 1], scale=1.0
                )
                # k = elu(k_raw * g) + 1 = min(exp(kg),1) + max(kg,0)
                kg = elup.tile([128, TB], F32, name="kg", tag="kg")
                nc.vector.tensor_mul(kg[:], kps[:], gsl)
                relu = elup.tile([128, TB], BF16, name="relu", tag="relu")
                nc.scalar.activation(relu[:], kg[:], AF.Relu)
                ex = elup.tile([128, TB], BF16, name="ex", tag="ex")
                nc.scalar.activation(ex[:], kg[:], AF.Exp)
                k_fm = elup.tile([128, TB], BF16, name="k_fm", tag="kfm", bufs=3)
                nc.vector.scalar_tensor_tensor(
                    k_fm[:],
                    in0=ex[:],
                    scalar=1.0,
                    in1=relu[:],
                    op0=ALU.min,
                    op1=ALU.add,
                )
                # to token-major via the DMA xbar (sync queue is otherwise
                # idle during phase 1)
                nc.sync.dma_start(kT4[:, :, m, :], k_fm[:], transpose=True)

            # v projection, token-major, augmented ones column per head
            vaugs = []
            for si in range(4):
                vps = [
                    mmps.tile([128, TB], F32, name=f"vps{nn}", tag="mm")
                    for nn in range(2)
                ]
                for k in range(KC):
                    lhsT = xs[g][:, k * TB + si * 128 : k * TB + (si + 1) * 128]
                    for nn in range(2):
                        nc.tensor.matmul(
                            vps[nn][:],
                            lhsT=lhsT,
                            rhs=wkv_sb[
                                :,
                                k * C2 + C + nn * TB : k * C2 + C + (nn + 1) * TB,
                            ],
                            start=(k == 0),
                            stop=(k == KC - 1),
                        )
                v_aug = workp.tile(
                    [128, H * 65], BF16, name="v_aug", tag="v_aug", bufs=4
                )
                v3 = v_aug.rearrange("p (h e) -> p h e", e=65)
                nc.vector.memset(v3[:, :, 64:65], 1.0)
                for nn in range(2):
                    h0 = nn * 8
                    nc.scalar.copy(
                        v3[:, h0 : h0 + 8, 0:64],
                        vps[nn].rearrange("p (h e) -> p h e", e=64),
                    )
                vaugs.append(v_aug)
            # kv einsum for this slab, head pairs packed [128, 130]:
            #   rows 0:64,  cols +0:65   = kv_aug head 2p   (col 64 = k_sum)
            #   rows 64:128, cols +65:130 = kv_aug head 2p+1 (col 129 = k_sum)
            for p in range(KC):
                eps = pnps.tile([128, TB], F32, name="eps", tag="pn")
                for si in range(4):
                    nc.tensor.matmul(
                        eps[:, 0:130],
                        lhsT=kT4[:, si, p, :],
                        rhs=vaugs[si][:, 130 * p : 130 * (p + 1)],
                        start=(si == 0),
                        stop=(si == 3),
                    )
                if g == 0:
                    nc.vector.tensor_copy(
                        kv_acc[:, 130 * p : 130 * (p + 1)], eps[:, 0:130]
                    )
                else:
                    nc.vector.tensor_add(
                        kv_acc[:, 130 * p : 130 * (p + 1)],
                        kv_acc[:, 130 * p : 130 * (p + 1)],
                        eps[:, 0:130],
                    )

        # ------------------------------------------------ kv all-reduce (pairs)
        # compact to [128, 8*65]: head 2p at [0:64, 65p:65p+65],
        # head 2p+1 at [64:128, 65p:65p+65]
        kv_cat = constp.tile([128, KC * 65], F32, name="kv_cat", tag="kvc")
        nc.vector.tensor_copy(
            kv_cat[0:64, :].rearrange("p (j e) -> p j e", e=65),
            kv_acc[0:64, :].rearrange("p (j q) -> p j q", q=130)[:, :, 0:65],
        )
        nc.vector.tensor_copy(
            kv_cat[64:128, :].rearrange("p (j e) -> p j e", e=65),
            kv_acc[64:128, :].rearrange("p (j q) -> p j q", q=130)[:, :, 65:130],
        )
        bounce_in = dramp.tile([128, KC * 65], F32, name="bounce_in")
        bounce_out = dramp.tile([128, KC * 65], F32, name="bounce_out")
        nc.sync.dma_start(bounce_in[:], kv_cat[:])
        nc.gpsimd.collective_compute(
            "AllReduce",
            ALU.add,
            replica_groups=REPLICA_GROUPS,
            ins=[bounce_in.opt()],
            outs=[bounce_out.opt()],
        )

        q_sb = wbig.tile([128, KC * T], BF16, name="q_sb", tag="wb", bufs=1)

        # ---------------------------------------- phase 1.5/2 (interleaved)
        def q_block(n):
            # q[m*128+c, tok] = elu(q_raw * g) + 1 for token block n
            for mh in range(2):
                qps = [
                    mmps.tile([128, TB], F32, name=f"qps{mi}", tag="mm")
                    for mi in range(4)
                ]
                for k in range(KC):
                    for mi in range(4):
                        m = mh * 4 + mi
                        nc.tensor.matmul(
                            qps[mi][:],
                            lhsT=wq_sb[:, k * C + m * 128 : k * C + (m + 1) * 128],
                            rhs=xs[n][:, k * TB : (k + 1) * TB],
                            start=(k == 0),
                            stop=(k == KC - 1),
                        )
                for mi in range(4):
                    m = mh * 4 + mi
                    qg = elup.tile([128, TB], F32, name="qg", tag="kg")
                    nc.vector.tensor_mul(
                        qg[:],
                        qps[mi][:],
                        gates_sb[:, m * T + n * TB : m * T + (n + 1) * TB],
                    )
                    relu = elup.tile([128, TB], BF16, name="relu2", tag="relu")
                    nc.scalar.activation(relu[:], qg[:], AF.Relu)
                    ex = elup.tile([128, TB], BF16, name="ex2", tag="ex")
                    nc.scalar.activation(ex[:], qg[:], AF.Exp)
                    nc.vector.scalar_tensor_tensor(
                        q_sb[:, m * T + n * TB : m * T + (n + 1) * TB],
                        in0=ex[:],
                        scalar=1.0,
                        in1=relu[:],
                        op0=ALU.min,
                        op1=ALU.add,
                    )

        def unpack_ar():
            # recv + build the attention lhsTs; emitted mid-q so the queues
            # absorb it right when the AR lands
            kv_f32 = constp.tile([128, KC * 65], F32, name="kv_f32", tag="kvc")
            nc.sync.dma_start(kv_f32[:], bounce_out[:])
            kv_bf = constp.tile([128, KC * 65], BF16, name="kv_bf")
            nc.vector.tensor_copy(kv_bf[:], kv_f32[:])
            # block-diagonal kv (K=128 per head pair j)
            bds_all = constp.tile([128, KC * 128], BF16, name="bds_all")
            nc.vector.memset(bds_all[:], 0.0)
            nc.vector.tensor_copy(
                bds_all[0:64, :].rearrange("p (j c) -> p j c", c=128)[:, :, 0:64],
                kv_bf[0:64, :].rearrange("p (j q) -> p j q", q=65)[:, :, 0:64],
            )
            nc.vector.tensor_copy(
                bds_all[64:128, :].rearrange("p (j c) -> p j c", c=128)[
                    :, :, 64:128
                ],
                kv_bf[64:128, :].rearrange("p (j q) -> p j q", q=65)[:, :, 0:64],
            )
            # normalizer lhsT: ksum of head h replicated across that head's
            # 64 output columns, so norm broadcasts straight out of the PE
            nb_all = constp.tile([128, KC * 128], BF16, name="nb_all")
            nc.vector.memset(nb_all[:], 0.0)
            for j in range(KC):
                nc.scalar.activation(
                    nb_all[0:64, j * 128 : j * 128 + 64],
                    kv_f32[0:64, 0:64],
                    AF.Identity,
                    bias=kv_f32[0:64, 65 * j + 64 : 65 * j + 65],
                    scale=0.0,
                )
                nc.scalar.activation(
                    nb_all[64:128, j * 128 + 64 : j * 128 + 128],
                    kv_f32[64:128, 0:64],
                    AF.Identity,
                    bias=kv_f32[64:128, 65 * j + 64 : 65 * j + 65],
                    scale=0.0,
                )
            return bds_all, nb_all

        def pn_block(n, bds_all, nb_all):
            # attn[e, tok] = (q @ kv) / (q . k_sum), feature-major
            attn = xsp.tile([128, KC * TB], BF16, name=f"attn{n}", tag="xs", bufs=4)
            for j in range(KC):
                pps = pnps.tile([128, TB], F32, name="pps", tag="pn")
                nc.tensor.matmul(
                    pps[:],
                    lhsT=bds_all[:, 128 * j : 128 * (j + 1)],
                    rhs=q_sb[:, j * T + n * TB : j * T + (n + 1) * TB],
                    start=True,
                    stop=True,
                )
                nrm = pnps.tile([128, TB], F32, name="nrm", tag="pn")
                nc.tensor.matmul(
                    nrm[:],
                    lhsT=nb_all[:, 128 * j : 128 * (j + 1)],
                    rhs=q_sb[:, j * T + n * TB : j * T + (n + 1) * TB],
                    start=True,
                    stop=True,
                )
                rec = elup.tile([128, TB], F32, name="rec", tag="rec")
                nc.vector.reciprocal_approx_fast(rec[:], nrm[:])
                nc.vector.tensor_mul(
                    attn[:, j * TB : (j + 1) * TB], pps[:], rec[:]
                )
            return attn

        def op_block(n, attn):
            # y[o, tok] = Wp^T @ attn + bp for token block n
            for m in range(KC):
                ops_ = mmps.tile([128, TB], F32, name="ops", tag="mm")
                for j in range(KC):
                    nc.tensor.matmul(
                        ops_[:],
                        lhsT=wp_sb[:, j * C + m * 128 : j * C + (m + 1) * 128],
                        rhs=attn[:, j * TB : (j + 1) * TB],
                        start=(j == 0),
                        stop=(j == KC - 1),
                    )
                o = outp.tile([128, TB], BF16, name="o", tag="o", bufs=4)
                nc.scalar.activation(
                    o[:],
                    ops_[:],
                    AF.Identity,
                    bias=bp_sb[:, m : m + 1],
                    scale=1.0,
                )
                nc.sync.dma_start(
                    y[128 * m : 128 * (m + 1), n * TB : (n + 1) * TB], o[:]
                )

        q_block(0)
        q_block(1)
        bds_all, nb_all = unpack_ar()
        a0 = pn_block(0, bds_all, nb_all)
        q_block(2)
        a1 = pn_block(1, bds_all, nb_all)
        op_block(0, a0)
        q_block(3)
        a2 = pn_block(2, bds_all, nb_all)
        op_block(1, a1)
        a3 = pn_block(3, bds_all, nb_all)
        op_block(2, a2)
        op_block(3, a3)


_NC_CACHE = {}


def get_nc():
    if "nc" not in _NC_CACHE:
        _NC_CACHE["nc"] = build_nc()
    return _NC_CACHE["nc"]


def make_in_maps(x, Wqkv, Wg, bg, Wp, bp):
    bf = ml_dtypes.bfloat16
    x = np.asarray(x, dtype=np.float32)
    Wqkv = np.asarray(Wqkv, dtype=np.float32)
    Wg = np.asarray(Wg, dtype=np.float32)
    bg = np.asarray(bg, dtype=np.float32)
    Wp = np.asarray(Wp, dtype=np.float32)
    bp = np.asarray(bp, dtype=np.float32)

    wq = np.ascontiguousarray(Wqkv[:, :C]).astype(bf)
    wkv = np.ascontiguousarray(Wqkv[:, C:]).astype(bf)
    wgt = Wg.astype(bf)
    wp = Wp.astype(bf)
    bg2 = np.ascontiguousarray(bg.reshape(KC, 128).T)
    bp2 = np.ascontiguousarray(bp.reshape(KC, 128).T)

    xf = x.reshape(NCORES, T, C)
    in_maps = []
    for c in range(NCORES):
        xtc = np.ascontiguousarray(xf[c].T).astype(bf)
        in_maps.append(
            dict(
                xt=xtc, wq=wq, wkv=wkv, wgt=wgt, wp=wp,
                bg2=bg2, bp2=bp2,
            )
        )
    return in_maps


def kernel(x, Wqkv, Wg, bg, Wp, bp, _collect_perf=None):
    nc = get_nc()
    in_maps = make_in_maps(x, Wqkv, Wg, bg, Wp, bp)
    kwargs = {}
    if _collect_perf is not None:
        kwargs = dict(trace=True)
        if _collect_perf.get("tmpdir"):
            kwargs["tmpdir"] = _collect_perf["tmpdir"]
    res = run_bass_kernel_spmd(
        nc, in_maps, core_ids=list(range(NCORES)), **kwargs
    )
    if _collect_perf is not None:
        _collect_perf["exec_time_ns"] = res.exec_time_ns
        _collect_perf["results"] = res
    out = np.empty((NCORES, T, C), dtype=np.float32)
    for c in range(NCORES):
        out[c] = res.results[c]["y"].astype(np.float32).T
    return out.reshape(B, N, C)


# revision 6
# speedup vs baseline: 1.1340x; 1.0354x over previous
"""Gated linear attention on 8 TRN2 NeuronCores.

Sharding: data-parallel over tokens. Core c handles tokens
[c*2048, (c+1)*2048) of the flattened (B*N, C) = (16384, 1024) sequence,
i.e. batch b = c//2, sequence half = c%2. The linear-attention kv state
(and k-sum) needs a reduction over each batch's full sequence, so cores
{2b, 2b+1} all-reduce a small (128, 520) fp32 buffer (kv state + k-sum
for 16 heads) and everything else is local.

Schedule notes (tuned against perfetto traces):
  - ALL weight/x loads issue in the first few us (weights on the gpsimd
    queue at ~25ns/issue, x slabs too) and complete by ~40us, so the
    all-reduce window has zero competing DMA traffic (big transfers
    during the collective delayed its trigger + RDMA sends by ~60us).
  - phase 1 streams per 512-token slab: gate-proj and k-proj both
    feature-major per m-chunk (sigmoid -> gate-mul -> elu fused per
    chunk, no cross-slab transpose barrier), k transposed to token-major
    AFTER the elu on the DMA xbar (sync queue), v token-major, then the
    kv einsum (PSUM per slab, DVE-accumulated into SBUF).
  - the AR bounce launches at phase-1 end; q-proj for the first two
    token blocks hides it. Attention normalizer uses a ksum-replicated
    lhsT so the matmul broadcasts norm[h, tok] to all 64 feature rows of
    head h directly (no separate broadcast matmul / PSUM-staging copy);
    reciprocal+multiply on DVE. PE emission order interleaves q blocks,
    attention blocks, and out-projection blocks to keep PE dense:
    q0 q1 [AR unpack] pn0 q2 pn1 op0 q3 pn2 op1 pn3 op2 op3.
"""

import numpy as np
import ml_dtypes

import concourse.bass as bass
import concourse.bacc as bacc
import concourse.tile as tile
import concourse.mybir as mybir
from concourse.bass_utils import run_bass_kernel_spmd

F32 = mybir.dt.float32
BF16 = mybir.dt.bfloat16
AF = mybir.ActivationFunctionType
ALU = mybir.AluOpType

B, N, C = 4, 4096, 1024
H, D = 16, 64
NCORES = 8
T = B * N // NCORES          # 2048 tokens per core
KC = C // 128                # 8 contraction chunks
TB = 512                     # token tile (free dim)
NT = T // TB                 # 4 token tiles / slabs
NS = T // 128                # 16 token subchunks (partition-dim tiles)
C2 = 2 * C

REPLICA_GROUPS = [[0, 1], [2, 3], [4, 5], [6, 7]]


def build_nc():
    nc = bacc.Bacc(
        "TRN2", target_bir_lowering=False, debug=False, num_devices=NCORES
    )
    xt = nc.dram_tensor("xt", [C, T], BF16, kind="ExternalInput")
    wq = nc.dram_tensor("wq", [C, C], BF16, kind="ExternalInput")
    wkv = nc.dram_tensor("wkv", [C, C2], BF16, kind="ExternalInput")
    wgt = nc.dram_tensor("wgt", [C, C], BF16, kind="ExternalInput")
    wp = nc.dram_tensor("wp", [C, C], BF16, kind="ExternalInput")
    bg2 = nc.dram_tensor("bg2", [128, KC], F32, kind="ExternalInput")
    bp2 = nc.dram_tensor("bp2", [128, KC], F32, kind="ExternalInput")
    y = nc.dram_tensor("y", [C, T], BF16, kind="ExternalOutput")

    with tile.TileContext(nc) as tc:
        build_body(nc, tc, xt, wq, wkv, wgt, wp, bg2, bp2, y)

    nc.compile()
    return nc


def build_body(nc, tc, xt, wq, wkv, wgt, wp, bg2, bp2, y):
    from contextlib import ExitStack

    with ExitStack() as st:
        constp = st.enter_context(tc.tile_pool(name="constp", bufs=1))
        wgp = st.enter_context(tc.tile_pool(name="wgp", bufs=1))
        wqp = st.enter_context(tc.tile_pool(name="wqp", bufs=1))
        wpp = st.enter_context(tc.tile_pool(name="wpp", bufs=1))
        wbig = st.enter_context(tc.tile_pool(name="wbig", bufs=1))
        xsp = st.enter_context(tc.tile_pool(name="xsp", bufs=4))
        gatesp = st.enter_context(tc.tile_pool(name="gatesp", bufs=1))
        ktp = st.enter_context(tc.tile_pool(name="ktp", bufs=2))
        workp = st.enter_context(tc.tile_pool(name="workp", bufs=4))
        elup = st.enter_context(tc.tile_pool(name="elup", bufs=2))
        outp = st.enter_context(tc.tile_pool(name="outp", bufs=4))
        mmps = st.enter_context(tc.tile_pool(name="mmps", bufs=5, space="PSUM"))
        pnps = st.enter_context(tc.tile_pool(name="pnps", bufs=3, space="PSUM"))
        dramp = st.enter_context(tc.tile_pool(name="dramp", bufs=1, space="DRAM"))

        # ------------------------------------------------ loads (all upfront)
        bg_sb = constp.tile([128, KC], F32, name="bg_sb")
        nc.scalar.dma_start(bg_sb[:], bg2[:])
        bp_sb = constp.tile([128, KC], F32, name="bp_sb")
        nc.scalar.dma_start(bp_sb[:], bp2[:])

        xs = []
        for g in range(NT):
            xg = xsp.tile([128, KC * TB], BF16, name=f"xs{g}", tag="xs", bufs=4)
            xs.append(xg)
        wg_sb = wgp.tile([128, KC * C], BF16, name="wg_sb")
        xt3 = xt.rearrange("(k p) t -> p k t", p=128)
        # wg + xs0 chunk-interleaved so the gates matmuls start ~2us in;
        # everything later as single big rearranged DMAs (cheap issue)
        for k in range(KC):
            nc.gpsimd.dma_start(
                wg_sb[:, k * C : (k + 1) * C], wgt[k * 128 : (k + 1) * 128, :]
            )
            nc.gpsimd.dma_start(
                xs[0][:, k * TB : (k + 1) * TB],
                xt[k * 128 : (k + 1) * 128, 0:TB],
            )
        wkv_sb = wbig.tile([128, KC * C2], BF16, name="wkv_sb", tag="wb", bufs=1)
        nc.gpsimd.dma_start(
            wkv_sb.rearrange("p (k n) -> p k n", k=KC),
            wkv.rearrange("(k p) n -> p k n", p=128),
        )
        for g in range(1, NT):
            nc.gpsimd.dma_start(
                xs[g].rearrange("p (k t) -> p k t", k=KC),
                xt3[:, :, g * TB : (g + 1) * TB],
            )
        wq_sb = wqp.tile([128, KC * C], BF16, name="wq_sb")
        nc.gpsimd.dma_start(
            wq_sb.rearrange("p (k n) -> p k n", k=KC),
            wq.rearrange("(k p) n -> p k n", p=128),
        )
        wp_sb = wpp.tile([128, KC * C], BF16, name="wp_sb")
        nc.gpsimd.dma_start(
            wp_sb.rearrange("p (k n) -> p k n", k=KC),
            wp.rearrange("(k p) n -> p k n", p=128),
        )

        gates_sb = gatesp.tile([128, KC * T], BF16, name="gates_sb", tag="g")
        kv_acc = constp.tile([128, KC * 130], F32, name="kv_acc")

        # -------------------------------- phase 1a: gates for all slabs
        # (one sigmoid act-table epoch; exp-table work only starts in 1b —
        # sigmoid and exp share no table, so interleaving them reloads a
        # 1.3us act table per switch)
        for g in range(NT):
            for mh in range(2):
                gps = [
                    mmps.tile([128, TB], F32, name=f"gps{mi}", tag="mm")
                    for mi in range(4)
                ]
                for k in range(KC):
                    for mi in range(4):
                        m = mh * 4 + mi
                        nc.tensor.matmul(
                            gps[mi][:],
                            lhsT=wg_sb[:, k * C + m * 128 : k * C + (m + 1) * 128],
                            rhs=xs[g][:, k * TB : (k + 1) * TB],
                            start=(k == 0),
                            stop=(k == KC - 1),
                        )
                for mi in range(4):
                    m = mh * 4 + mi
                    nc.scalar.activation(
                        gates_sb[:, m * T + g * TB : m * T + (g + 1) * TB],
                        gps[mi][:],
                        AF.Sigmoid,
                        bias=bg_sb[:, m : m + 1],
                        scale=1.0,
                    )

        # -------------------------------- phase 1b: k/v + kv state per slab
        for g in range(NT):
            # k^T for this slab, token-major: kT4[p, si, m, c] =
            # k[feat m*128+c, tok si*128+p]
            k_T = ktp.tile([128, 4 * KC * 128], BF16, name="k_T", tag="kt", bufs=2)
            kT4 = k_T.rearrange("p (s m c) -> p s m c", s=4, c=128)
            for m in range(KC):
                kps = mmps.tile([128, TB], F32, name="kps", tag="mm")
                for k in range(KC):
                    nc.tensor.matmul(
                        kps[:],
                        lhsT=wkv_sb[
                            :, k * C2 + m * 128 : k * C2 + (m + 1) * 128
                        ],
                        rhs=xs[g][:, k * TB : (k + 1) * TB],
                        start=(k == 0),
                        stop=(k == KC - 1),
                    )
                gsl = gates_sb[:, m * T + g * TB : m * T + (g + 1) * TB]
                # k = elu(k_raw * g) + 1 = min(exp(kg),1) + max(kg,0)
                kg = elup.tile([128, TB], F32, name="kg", tag="kg")
                nc.vector.tensor_mul(kg[:], kps[:], gsl)
                relu = elup.tile([128, TB], BF16, name="relu", tag="relu")
                nc.scalar.activation(relu[:], kg[:], AF.Relu)
                ex = elup.tile([128, TB], BF16, name="ex", tag="ex")
                nc.scalar.activation(ex[:], kg[:], AF.Exp)
                k_fm = elup.tile([128, TB], BF16, name="k_fm", tag="kfm", bufs=3)
                nc.vector.scalar_tensor_tensor(
                    k_fm[:],
                    in0=ex[:],
                    scalar=1.0,
                    in1=relu[:],
                    op0=ALU.min,
                    op1=ALU.add,
                )
                # to token-major via the DMA xbar (sync queue is otherwise
                # idle during phase 1)
                nc.sync.dma_start(kT4[:, :, m, :], k_fm[:], transpose=True)

            # v projection, token-major, augmented ones column per head
            vaugs = []
            for si in range(4):
                vps = [
                    mmps.tile([128, TB], F32, name=f"vps{nn}", tag="mm")
                    for nn in range(2)
                ]
                for k in range(KC):
                    lhsT = xs[g][:, k * TB + si * 128 : k * TB + (si + 1) * 128]
                    for nn in range(2):
                        nc.tensor.matmul(
                            vps[nn][:],
                            lhsT=lhsT,
                            rhs=wkv_sb[
                                :,
                                k * C2 + C + nn * TB : k * C2 + C + (nn + 1) * TB,
                            ],
                            start=(k == 0),
                            stop=(k == KC - 1),
                        )
                v_aug = workp.tile(
                    [128, H * 65], BF16, name="v_aug", tag="v_aug", bufs=4
                )
                v3 = v_aug.rearrange("p (h e) -> p h e", e=65)
                nc.vector.memset(v3[:, :, 64:65], 1.0)
                for nn in range(2):
                    h0 = nn * 8
                    nc.scalar.copy(
                        v3[:, h0 : h0 + 8, 0:64],
                        vps[nn].rearrange("p (h e) -> p h e", e=64),
                    )
                vaugs.append(v_aug)
            # kv einsum for this slab, head pairs packed [128, 130]:
            #   rows 0:64,  cols +0:65   = kv_aug head 2p   (col 64 = k_sum)
            #   rows 64:128, cols +65:130 = kv_aug head 2p+1 (col 129 = k_sum)
            for p in range(KC):
                eps = pnps.tile([128, TB], F32, name="eps", tag="pn")
                for si in range(4):
                    nc.tensor.matmul(
                        eps[:, 0:130],
                        lhsT=kT4[:, si, p, :],
                        rhs=vaugs[si][:, 130 * p : 130 * (p + 1)],
                        start=(si == 0),
                        stop=(si == 3),
                    )
                if g == 0:
                    nc.vector.tensor_copy(
                        kv_acc[:, 130 * p : 130 * (p + 1)], eps[:, 0:130]
                    )
                else:
                    nc.vector.tensor_add(
                        kv_acc[:, 130 * p : 130 * (p + 1)],
                        kv_acc[:, 130 * p : 130 * (p + 1)],
                        eps[:, 0:130],
                    )

        # ------------------------------------------------ kv all-reduce (pairs)
        # compact to [128, 8*65]: head 2p at [0:64, 65p:65p+65],
        # head 2p+1 at [64:128, 65p:65p+65]
        kv_cat = constp.tile([128, KC * 65], F32, name="kv_cat", tag="kvc")
        nc.vector.tensor_copy(
            kv_cat[0:64, :].rearrange("p (j e) -> p j e", e=65),
            kv_acc[0:64, :].rearrange("p (j q) -> p j q", q=130)[:, :, 0:65],
        )
        nc.vector.tensor_copy(
            kv_cat[64:128, :].rearrange("p (j e) -> p j e", e=65),
            kv_acc[64:128, :].rearrange("p (j q) -> p j q", q=130)[:, :, 65:130],
        )
        bounce_in = dramp.tile([128, KC * 65], F32, name="bounce_in")
        bounce_out = dramp.tile([128, KC * 65], F32, name="bounce_out")
        nc.sync.dma_start(bounce_in[:], kv_cat[:])
        nc.gpsimd.collective_compute(
            "AllReduce",
            ALU.add,
            replica_groups=REPLICA_GROUPS,
            ins=[bounce_in.opt()],
            outs=[bounce_out.opt()],
        )

        q_sb = wbig.tile([128, KC * T], BF16, name="q_sb", tag="wb", bufs=1)

        # ---------------------------------------- phase 1.5/2 (interleaved)
        def q_block(n):
            # q[m*128+c, tok] = elu(q_raw * g) + 1 for token block n
            for mh in range(2):
                qps = [
                    mmps.tile([128, TB], F32, name=f"qps{mi}", tag="mm")
                    for mi in range(4)
                ]
                for k in range(KC):
                    for mi in range(4):
                        m = mh * 4 + mi
                        nc.tensor.matmul(
                            qps[mi][:],
                            lhsT=wq_sb[:, k * C + m * 128 : k * C + (m + 1) * 128],
                            rhs=xs[n][:, k * TB : (k + 1) * TB],
                            start=(k == 0),
                            stop=(k == KC - 1),
                        )
                for mi in range(4):
                    m = mh * 4 + mi
                    qg = elup.tile([128, TB], F32, name="qg", tag="kg")
                    nc.vector.tensor_mul(
                        qg[:],
                        qps[mi][:],
                        gates_sb[:, m * T + n * TB : m * T + (n + 1) * TB],
                    )
                    relu = elup.tile([128, TB], BF16, name="relu2", tag="relu")
                    nc.scalar.activation(relu[:], qg[:], AF.Relu)
                    ex = elup.tile([128, TB], BF16, name="ex2", tag="ex")
                    nc.scalar.activation(ex[:], qg[:], AF.Exp)
                    nc.vector.scalar_tensor_tensor(
                        q_sb[:, m * T + n * TB : m * T + (n + 1) * TB],
                        in0=ex[:],
                        scalar=1.0,
                        in1=relu[:],
                        op0=ALU.min,
                        op1=ALU.add,
                    )

        def unpack_ar():
            # recv + build the attention lhsTs; emitted mid-q so the queues
            # absorb it right when the AR lands
            kv_f32 = constp.tile([128, KC * 65], F32, name="kv_f32", tag="kvc")
            nc.sync.dma_start(kv_f32[:], bounce_out[:])
            kv_bf = constp.tile([128, KC * 65], BF16, name="kv_bf")
            nc.vector.tensor_copy(kv_bf[:], kv_f32[:])
            # block-diagonal kv (K=128 per head pair j)
            bds_all = constp.tile([128, KC * 128], BF16, name="bds_all")
            nc.vector.memset(bds_all[:], 0.0)
            nc.vector.tensor_copy(
                bds_all[0:64, :].rearrange("p (j c) -> p j c", c=128)[:, :, 0:64],
                kv_bf[0:64, :].rearrange("p (j q) -> p j q", q=65)[:, :, 0:64],
            )
            nc.vector.tensor_copy(
                bds_all[64:128, :].rearrange("p (j c) -> p j c", c=128)[
                    :, :, 64:128
                ],
                kv_bf[64:128, :].rearrange("p (j q) -> p j q", q=65)[:, :, 0:64],
            )
            # normalizer lhsT: ksum of head h replicated across that head's
            # 64 output columns, so norm broadcasts straight out of the PE
            nb_all = constp.tile([128, KC * 128], BF16, name="nb_all")
            nc.vector.memset(nb_all[:], 0.0)
            for j in range(KC):
                nc.scalar.activation(
                    nb_all[0:64, j * 128 : j * 128 + 64],
                    kv_f32[0:64, 0:64],
                    AF.Identity,
                    bias=kv_f32[0:64, 65 * j + 64 : 65 * j + 65],
                    scale=0.0,
                )
                nc.scalar.activation(
                    nb_all[64:128, j * 128 + 64 : j * 128 + 128],
                    kv_f32[64:128, 0:64],
                    AF.Identity,
                    bias=kv_f32[64:128, 65 * j + 64 : 65 * j + 65],
                    scale=0.0,
                )
            return bds_all, nb_all

        def pn_block(n, bds_all, nb_all):
            # attn[e, tok] = (q @ kv) / (q . k_sum), feature-major
            attn = xsp.tile([128, KC * TB], BF16, name=f"attn{n}", tag="xs", bufs=4)
            for j in range(KC):
                pps = pnps.tile([128, TB], F32, name="pps", tag="pn")
                nc.tensor.matmul(
                    pps[:],
                    lhsT=bds_all[:, 128 * j : 128 * (j + 1)],
                    rhs=q_sb[:, j * T + n * TB : j * T + (n + 1) * TB],
                    start=True,
                    stop=True,
                )
                nrm = pnps.tile([128, TB], F32, name="nrm", tag="pn")
                nc.tensor.matmul(
                    nrm[:],
                    lhsT=nb_all[:, 128 * j : 128 * (j + 1)],
                    rhs=q_sb[:, j * T + n * TB : j * T + (n + 1) * TB],
                    start=True,
                    stop=True,
                )
                rec = elup.tile([128, TB], F32, name="rec", tag="rec")
                nc.vector.reciprocal_approx_fast(rec[:], nrm[:])
                nc.vector.tensor_mul(
                    attn[:, j * TB : (j + 1) * TB], pps[:], rec[:]
                )
            return attn

        y3 = y.rearrange("(m p) t -> p m t", p=128)

        def op_block(n, attn):
            # y[o, tok] = Wp^T @ attn + bp for token block n; stores batched
            # 4 m-chunks per DMA so the tail isn't paced by per-chunk stores
            for mg in range(2):
                o4 = outp.tile([128, 4 * TB], BF16, name="o4", tag="o", bufs=2)
                for mi in range(4):
                    m = mg * 4 + mi
                    ops_ = mmps.tile([128, TB], F32, name="ops", tag="mm")
                    for j in range(KC):
                        nc.tensor.matmul(
                            ops_[:],
                            lhsT=wp_sb[:, j * C + m * 128 : j * C + (m + 1) * 128],
                            rhs=attn[:, j * TB : (j + 1) * TB],
                            start=(j == 0),
                            stop=(j == KC - 1),
                        )
                    nc.scalar.activation(
                        o4[:, mi * TB : (mi + 1) * TB],
                        ops_[:],
                        AF.Identity,
                        bias=bp_sb[:, m : m + 1],
                        scale=1.0,
                    )
                nc.sync.dma_start(
                    y3[:, mg * 4 : (mg + 1) * 4, n * TB : (n + 1) * TB],
                    o4.rearrange("p (m t) -> p m t", t=TB),
                )

        q_block(0)
        q_block(1)
        bds_all, nb_all = unpack_ar()
        a0 = pn_block(0, bds_all, nb_all)
        q_block(2)
        a1 = pn_block(1, bds_all, nb_all)
        op_block(0, a0)
        q_block(3)
        a2 = pn_block(2, bds_all, nb_all)
        op_block(1, a1)
        a3 = pn_block(3, bds_all, nb_all)
        op_block(2, a2)
        op_block(3, a3)


_NC_CACHE = {}


def get_nc():
    if "nc" not in _NC_CACHE:
        _NC_CACHE["nc"] = build_nc()
    return _NC_CACHE["nc"]


def make_in_maps(x, Wqkv, Wg, bg, Wp, bp):
    bf = ml_dtypes.bfloat16
    x = np.asarray(x, dtype=np.float32)
    Wqkv = np.asarray(Wqkv, dtype=np.float32)
    Wg = np.asarray(Wg, dtype=np.float32)
    bg = np.asarray(bg, dtype=np.float32)
    Wp = np.asarray(Wp, dtype=np.float32)
    bp = np.asarray(bp, dtype=np.float32)

    wq = np.ascontiguousarray(Wqkv[:, :C]).astype(bf)
    wkv = np.ascontiguousarray(Wqkv[:, C:]).astype(bf)
    wgt = Wg.astype(bf)
    wp = Wp.astype(bf)
    bg2 = np.ascontiguousarray(bg.reshape(KC, 128).T)
    bp2 = np.ascontiguousarray(bp.reshape(KC, 128).T)

    xf = x.reshape(NCORES, T, C)
    in_maps = []
    for c in range(NCORES):
        xtc = np.ascontiguousarray(xf[c].T).astype(bf)
        in_maps.append(
            dict(
                xt=xtc, wq=wq, wkv=wkv, wgt=wgt, wp=wp,
                bg2=bg2, bp2=bp2,
            )
        )
    return in_maps


def kernel(x, Wqkv, Wg, bg, Wp, bp, _collect_perf=None):
    nc = get_nc()
    in_maps = make_in_maps(x, Wqkv, Wg, bg, Wp, bp)
    kwargs = {}
    if _collect_perf is not None:
        kwargs = dict(trace=True)
        if _collect_perf.get("tmpdir"):
            kwargs["tmpdir"] = _collect_perf["tmpdir"]
    res = run_bass_kernel_spmd(
        nc, in_maps, core_ids=list(range(NCORES)), **kwargs
    )
    if _collect_perf is not None:
        _collect_perf["exec_time_ns"] = res.exec_time_ns
        _collect_perf["results"] = res
    out = np.empty((NCORES, T, C), dtype=np.float32)
    for c in range(NCORES):
        out[c] = res.results[c]["y"].astype(np.float32).T
    return out.reshape(B, N, C)


# revision 10
# speedup vs baseline: 1.1666x; 1.0288x over previous
"""Gated linear attention on 8 TRN2 NeuronCores.

Sharding: data-parallel over tokens. Core c handles tokens
[c*2048, (c+1)*2048) of the flattened (B*N, C) = (16384, 1024) sequence,
i.e. batch b = c//2, sequence half = c%2. The linear-attention kv state
(and k-sum) needs a reduction over each batch's full sequence, so cores
{2b, 2b+1} all-reduce a small (128, 520) fp32 buffer (kv state + k-sum
for 16 heads) and everything else is local.

Schedule notes (tuned against perfetto traces):
  - ALL weight/x loads issue in the first few us (weights on the gpsimd
    queue at ~25ns/issue, x slabs too) and complete by ~40us, so the
    all-reduce window has zero competing DMA traffic (big transfers
    during the collective delayed its trigger + RDMA sends by ~60us).
  - phase 1 streams per 512-token slab: gate-proj and k-proj both
    feature-major per m-chunk (sigmoid -> gate-mul -> elu fused per
    chunk, no cross-slab transpose barrier), k transposed to token-major
    AFTER the elu on the DMA xbar (sync queue), v token-major, then the
    kv einsum (PSUM per slab, DVE-accumulated into SBUF).
  - the AR bounce launches at phase-1 end; q-proj for the first two
    token blocks hides it. Attention normalizer uses a ksum-replicated
    lhsT so the matmul broadcasts norm[h, tok] to all 64 feature rows of
    head h directly (no separate broadcast matmul / PSUM-staging copy);
    reciprocal+multiply on DVE. PE emission order interleaves q blocks,
    attention blocks, and out-projection blocks to keep PE dense:
    q0 q1 [AR unpack] pn0 q2 pn1 op0 q3 pn2 op1 pn3 op2 op3.
"""

import numpy as np
import ml_dtypes

import concourse.bass as bass
import concourse.bacc as bacc
import concourse.tile as tile
import concourse.mybir as mybir
from concourse.bass_utils import run_bass_kernel_spmd

F32 = mybir.dt.float32
BF16 = mybir.dt.bfloat16
AF = mybir.ActivationFunctionType
ALU = mybir.AluOpType

B, N, C = 4, 4096, 1024
H, D = 16, 64
NCORES = 8
T = B * N // NCORES          # 2048 tokens per core
KC = C // 128                # 8 contraction chunks
TB = 512                     # token tile (free dim)
NT = T // TB                 # 4 token tiles / slabs
NS = T // 128                # 16 token subchunks (partition-dim tiles)
C2 = 2 * C

REPLICA_GROUPS = [[0, 1], [2, 3], [4, 5], [6, 7]]


def build_nc():
    nc = bacc.Bacc(
        "TRN2", target_bir_lowering=False, debug=False, num_devices=NCORES
    )
    xt = nc.dram_tensor("xt", [C, T], BF16, kind="ExternalInput")
    wq = nc.dram_tensor("wq", [C, C], BF16, kind="ExternalInput")
    wkv = nc.dram_tensor("wkv", [C, C2], BF16, kind="ExternalInput")
    wgt = nc.dram_tensor("wgt", [C, C], BF16, kind="ExternalInput")
    wp = nc.dram_tensor("wp", [C, C], BF16, kind="ExternalInput")
    bg2 = nc.dram_tensor("bg2", [128, KC], F32, kind="ExternalInput")
    bp2 = nc.dram_tensor("bp2", [128, KC], F32, kind="ExternalInput")
    y = nc.dram_tensor("y", [C, T], BF16, kind="ExternalOutput")

    with tile.TileContext(nc) as tc:
        build_body(nc, tc, xt, wq, wkv, wgt, wp, bg2, bp2, y)

    nc.compile()
    return nc


def build_body(nc, tc, xt, wq, wkv, wgt, wp, bg2, bp2, y):
    from contextlib import ExitStack

    with ExitStack() as st:
        constp = st.enter_context(tc.tile_pool(name="constp", bufs=1))
        wgp = st.enter_context(tc.tile_pool(name="wgp", bufs=1))
        wqp = st.enter_context(tc.tile_pool(name="wqp", bufs=1))
        wpp = st.enter_context(tc.tile_pool(name="wpp", bufs=1))
        wbig = st.enter_context(tc.tile_pool(name="wbig", bufs=1))
        xsp = st.enter_context(tc.tile_pool(name="xsp", bufs=4))
        gatesp = st.enter_context(tc.tile_pool(name="gatesp", bufs=1))
        ktp = st.enter_context(tc.tile_pool(name="ktp", bufs=2))
        workp = st.enter_context(tc.tile_pool(name="workp", bufs=4))
        elup = st.enter_context(tc.tile_pool(name="elup", bufs=2))
        outp = st.enter_context(tc.tile_pool(name="outp", bufs=4))
        mmps = st.enter_context(tc.tile_pool(name="mmps", bufs=5, space="PSUM"))
        pnps = st.enter_context(tc.tile_pool(name="pnps", bufs=3, space="PSUM"))
        dramp = st.enter_context(tc.tile_pool(name="dramp", bufs=1, space="DRAM"))

        # ------------------------------------------------ loads (all upfront)
        bg_sb = constp.tile([128, KC], F32, name="bg_sb")
        nc.scalar.dma_start(bg_sb[:], bg2[:])
        bp_sb = constp.tile([128, KC], F32, name="bp_sb")
        nc.scalar.dma_start(bp_sb[:], bp2[:])

        xs = []
        for g in range(NT):
            xg = xsp.tile([128, KC * TB], BF16, name=f"xs{g}", tag="xs", bufs=4)
            xs.append(xg)
        wg_sb = wgp.tile([128, KC * C], BF16, name="wg_sb")
        xt3 = xt.rearrange("(k p) t -> p k t", p=128)
        # wg + xs0 chunk-interleaved so the gates matmuls start ~2us in;
        # everything later as single big rearranged DMAs (cheap issue)
        for k in range(KC):
            nc.gpsimd.dma_start(
                wg_sb[:, k * C : (k + 1) * C], wgt[k * 128 : (k + 1) * 128, :]
            )
            nc.gpsimd.dma_start(
                xs[0][:, k * TB : (k + 1) * TB],
                xt[k * 128 : (k + 1) * 128, 0:TB],
            )
        for g in range(1, NT):
            nc.gpsimd.dma_start(
                xs[g].rearrange("p (k t) -> p k t", k=KC),
                xt3[:, :, g * TB : (g + 1) * TB],
            )
        wkv_sb = wbig.tile([128, KC * C2], BF16, name="wkv_sb", tag="wb", bufs=1)
        nc.gpsimd.dma_start(
            wkv_sb.rearrange("p (k n) -> p k n", k=KC),
            wkv.rearrange("(k p) n -> p k n", p=128),
        )
        wq_sb = wqp.tile([128, KC * C], BF16, name="wq_sb")
        nc.gpsimd.dma_start(
            wq_sb.rearrange("p (k n) -> p k n", k=KC),
            wq.rearrange("(k p) n -> p k n", p=128),
        )
        wp_sb = wpp.tile([128, KC * C], BF16, name="wp_sb")
        nc.gpsimd.dma_start(
            wp_sb.rearrange("p (k n) -> p k n", k=KC),
            wp.rearrange("(k p) n -> p k n", p=128),
        )

        gates_sb = gatesp.tile([128, KC * T], BF16, name="gates_sb", tag="g")
        kv_acc = constp.tile([128, KC * 130], F32, name="kv_acc")

        # -------------------------------- phase 1a: gates for all slabs
        # (one sigmoid act-table epoch; exp-table work only starts in 1b —
        # sigmoid and exp share no table, so interleaving them reloads a
        # 1.3us act table per switch)
        for g in range(NT):
            for mh in range(2):
                gps = [
                    mmps.tile([128, TB], F32, name=f"gps{mi}", tag="mm")
                    for mi in range(4)
                ]
                for k in range(KC):
                    for mi in range(4):
                        m = mh * 4 + mi
                        nc.tensor.matmul(
                            gps[mi][:],
                            lhsT=wg_sb[:, k * C + m * 128 : k * C + (m + 1) * 128],
                            rhs=xs[g][:, k * TB : (k + 1) * TB],
                            start=(k == 0),
                            stop=(k == KC - 1),
                        )
                for mi in range(4):
                    m = mh * 4 + mi
                    nc.scalar.activation(
                        gates_sb[:, m * T + g * TB : m * T + (g + 1) * TB],
                        gps[mi][:],
                        AF.Sigmoid,
                        bias=bg_sb[:, m : m + 1],
                        scale=1.0,
                    )

        # -------------------------------- phase 1b: k/v + kv state per slab
        for g in range(NT):
            # k^T for this slab, token-major: kT4[p, si, m, c] =
            # k[feat m*128+c, tok si*128+p]
            k_T = ktp.tile([128, 4 * KC * 128], BF16, name="k_T", tag="kt", bufs=2)
            kT4 = k_T.rearrange("p (s m c) -> p s m c", s=4, c=128)
            for m in range(KC):
                kps = mmps.tile([128, TB], F32, name="kps", tag="mm")
                for k in range(KC):
                    nc.tensor.matmul(
                        kps[:],
                        lhsT=wkv_sb[
                            :, k * C2 + m * 128 : k * C2 + (m + 1) * 128
                        ],
                        rhs=xs[g][:, k * TB : (k + 1) * TB],
                        start=(k == 0),
                        stop=(k == KC - 1),
                    )
                gsl = gates_sb[:, m * T + g * TB : m * T + (g + 1) * TB]
                # k = elu(k_raw * g) + 1 = min(exp(kg),1) + max(kg,0)
                kg = elup.tile([128, TB], F32, name="kg", tag="kg")
                nc.vector.tensor_mul(kg[:], kps[:], gsl)
                relu = elup.tile([128, TB], BF16, name="relu", tag="relu")
                nc.scalar.activation(relu[:], kg[:], AF.Relu)
                ex = elup.tile([128, TB], BF16, name="ex", tag="ex")
                nc.scalar.activation(ex[:], kg[:], AF.Exp)
                k_fm = elup.tile([128, TB], BF16, name="k_fm", tag="kfm", bufs=3)
                nc.vector.scalar_tensor_tensor(
                    k_fm[:],
                    in0=ex[:],
                    scalar=1.0,
                    in1=relu[:],
                    op0=ALU.min,
                    op1=ALU.add,
                )
                # to token-major via the DMA xbar (sync queue is otherwise
                # idle during phase 1)
                nc.sync.dma_start(kT4[:, :, m, :], k_fm[:], transpose=True)

            # v projection, token-major, augmented ones column per head
            vaugs = []
            for si in range(4):
                vps = [
                    mmps.tile([128, TB], F32, name=f"vps{nn}", tag="mm")
                    for nn in range(2)
                ]
                for k in range(KC):
                    lhsT = xs[g][:, k * TB + si * 128 : k * TB + (si + 1) * 128]
                    for nn in range(2):
                        nc.tensor.matmul(
                            vps[nn][:],
                            lhsT=lhsT,
                            rhs=wkv_sb[
                                :,
                                k * C2 + C + nn * TB : k * C2 + C + (nn + 1) * TB,
                            ],
                            start=(k == 0),
                            stop=(k == KC - 1),
                        )
                v_aug = workp.tile(
                    [128, H * 65], BF16, name="v_aug", tag="v_aug", bufs=4
                )
                v3 = v_aug.rearrange("p (h e) -> p h e", e=65)
                nc.vector.memset(v3[:, :, 64:65], 1.0)
                for nn in range(2):
                    h0 = nn * 8
                    nc.scalar.copy(
                        v3[:, h0 : h0 + 8, 0:64],
                        vps[nn].rearrange("p (h e) -> p h e", e=64),
                    )
                vaugs.append(v_aug)
            # kv einsum for this slab, head pairs packed [128, 130]:
            #   rows 0:64,  cols +0:65   = kv_aug head 2p   (col 64 = k_sum)
            #   rows 64:128, cols +65:130 = kv_aug head 2p+1 (col 129 = k_sum)
            for p in range(KC):
                eps = pnps.tile([128, TB], F32, name="eps", tag="pn")
                for si in range(4):
                    nc.tensor.matmul(
                        eps[:, 0:130],
                        lhsT=kT4[:, si, p, :],
                        rhs=vaugs[si][:, 130 * p : 130 * (p + 1)],
                        start=(si == 0),
                        stop=(si == 3),
                    )
                if g == 0:
                    nc.vector.tensor_copy(
                        kv_acc[:, 130 * p : 130 * (p + 1)], eps[:, 0:130]
                    )
                else:
                    nc.vector.tensor_add(
                        kv_acc[:, 130 * p : 130 * (p + 1)],
                        kv_acc[:, 130 * p : 130 * (p + 1)],
                        eps[:, 0:130],
                    )

        # ------------------------------------------------ kv all-reduce (pairs)
        # compact to [128, 8*65]: head 2p at [0:64, 65p:65p+65],
        # head 2p+1 at [64:128, 65p:65p+65]
        kv_cat = constp.tile([128, KC * 65], F32, name="kv_cat", tag="kvc")
        nc.vector.tensor_copy(
            kv_cat[0:64, :].rearrange("p (j e) -> p j e", e=65),
            kv_acc[0:64, :].rearrange("p (j q) -> p j q", q=130)[:, :, 0:65],
        )
        nc.vector.tensor_copy(
            kv_cat[64:128, :].rearrange("p (j e) -> p j e", e=65),
            kv_acc[64:128, :].rearrange("p (j q) -> p j q", q=130)[:, :, 65:130],
        )
        bounce_in = dramp.tile([128, KC * 65], F32, name="bounce_in")
        bounce_out = dramp.tile([128, KC * 65], F32, name="bounce_out")
        nc.sync.dma_start(bounce_in[:], kv_cat[:])
        nc.gpsimd.collective_compute(
            "AllReduce",
            ALU.add,
            replica_groups=REPLICA_GROUPS,
            ins=[bounce_in.opt()],
            outs=[bounce_out.opt()],
        )

        q_sb = wbig.tile([128, KC * T], BF16, name="q_sb", tag="wb", bufs=1)

        # ---------------------------------------- phase 1.5/2 (interleaved)
        def q_block(n, halves=(0, 1)):
            # q[m*128+c, tok] = elu(q_raw * g) + 1 for token block n
            for mh in halves:
                qps = [
                    mmps.tile([128, TB], F32, name=f"qps{mi}", tag="mm")
                    for mi in range(4)
                ]
                for k in range(KC):
                    for mi in range(4):
                        m = mh * 4 + mi
                        nc.tensor.matmul(
                            qps[mi][:],
                            lhsT=wq_sb[:, k * C + m * 128 : k * C + (m + 1) * 128],
                            rhs=xs[n][:, k * TB : (k + 1) * TB],
                            start=(k == 0),
                            stop=(k == KC - 1),
                        )
                for mi in range(4):
                    m = mh * 4 + mi
                    qg = elup.tile([128, TB], F32, name="qg", tag="kg")
                    nc.vector.tensor_mul(
                        qg[:],
                        qps[mi][:],
                        gates_sb[:, m * T + n * TB : m * T + (n + 1) * TB],
                    )
                    relu = elup.tile([128, TB], BF16, name="relu2", tag="relu")
                    nc.scalar.activation(relu[:], qg[:], AF.Relu)
                    ex = elup.tile([128, TB], BF16, name="ex2", tag="ex")
                    nc.scalar.activation(ex[:], qg[:], AF.Exp)
                    nc.vector.scalar_tensor_tensor(
                        q_sb[:, m * T + n * TB : m * T + (n + 1) * TB],
                        in0=ex[:],
                        scalar=1.0,
                        in1=relu[:],
                        op0=ALU.min,
                        op1=ALU.add,
                    )

        def unpack_ar():
            # recv + build the attention lhsTs; emitted mid-q so the queues
            # absorb it right when the AR lands
            kv_f32 = constp.tile([128, KC * 65], F32, name="kv_f32", tag="kvc")
            nc.sync.dma_start(kv_f32[:], bounce_out[:])
            kv_bf = constp.tile([128, KC * 65], BF16, name="kv_bf")
            nc.vector.tensor_copy(kv_bf[:], kv_f32[:])
            # block-diagonal kv (K=128 per head pair j)
            bds_all = constp.tile([128, KC * 128], BF16, name="bds_all")
            nc.vector.memset(bds_all[:], 0.0)
            nc.vector.tensor_copy(
                bds_all[0:64, :].rearrange("p (j c) -> p j c", c=128)[:, :, 0:64],
                kv_bf[0:64, :].rearrange("p (j q) -> p j q", q=65)[:, :, 0:64],
            )
            nc.vector.tensor_copy(
                bds_all[64:128, :].rearrange("p (j c) -> p j c", c=128)[
                    :, :, 64:128
                ],
                kv_bf[64:128, :].rearrange("p (j q) -> p j q", q=65)[:, :, 0:64],
            )
            # normalizer lhsT: ksum of head h replicated across that head's
            # 64 output columns, so norm broadcasts straight out of the PE
            nb_all = constp.tile([128, KC * 128], BF16, name="nb_all")
            nc.vector.memset(nb_all[:], 0.0)
            for j in range(KC):
                nc.scalar.activation(
                    nb_all[0:64, j * 128 : j * 128 + 64],
                    kv_f32[0:64, 0:64],
                    AF.Identity,
                    bias=kv_f32[0:64, 65 * j + 64 : 65 * j + 65],
                    scale=0.0,
                )
                nc.scalar.activation(
                    nb_all[64:128, j * 128 + 64 : j * 128 + 128],
                    kv_f32[64:128, 0:64],
                    AF.Identity,
                    bias=kv_f32[64:128, 65 * j + 64 : 65 * j + 65],
                    scale=0.0,
                )
            return bds_all, nb_all

        def pn_block(n, bds_all, nb_all):
            # attn[e, tok] = (q @ kv) / (q . k_sum), feature-major
            attn = xsp.tile([128, KC * TB], BF16, name=f"attn{n}", tag="xs", bufs=4)
            for j in range(KC):
                pps = pnps.tile([128, TB], F32, name="pps", tag="pn")
                nc.tensor.matmul(
                    pps[:],
                    lhsT=bds_all[:, 128 * j : 128 * (j + 1)],
                    rhs=q_sb[:, j * T + n * TB : j * T + (n + 1) * TB],
                    start=True,
                    stop=True,
                )
                nrm = pnps.tile([128, TB], F32, name="nrm", tag="pn")
                nc.tensor.matmul(
                    nrm[:],
                    lhsT=nb_all[:, 128 * j : 128 * (j + 1)],
                    rhs=q_sb[:, j * T + n * TB : j * T + (n + 1) * TB],
                    start=True,
                    stop=True,
                )
                rec = elup.tile([128, TB], F32, name="rec", tag="rec")
                nc.vector.reciprocal_approx_fast(rec[:], nrm[:])
                nc.vector.tensor_mul(
                    attn[:, j * TB : (j + 1) * TB], pps[:], rec[:]
                )
            return attn

        y3 = y.rearrange("(m p) t -> p m t", p=128)

        def op_block(n, attn):
            # y[o, tok] = Wp^T @ attn + bp for token block n; stores batched
            # 4 m-chunks per DMA so the tail isn't paced by per-chunk stores
            for mg in range(2):
                o4 = outp.tile([128, 4 * TB], BF16, name="o4", tag="o", bufs=2)
                for mi in range(4):
                    m = mg * 4 + mi
                    ops_ = mmps.tile([128, TB], F32, name="ops", tag="mm")
                    for j in range(KC):
                        nc.tensor.matmul(
                            ops_[:],
                            lhsT=wp_sb[:, j * C + m * 128 : j * C + (m + 1) * 128],
                            rhs=attn[:, j * TB : (j + 1) * TB],
                            start=(j == 0),
                            stop=(j == KC - 1),
                        )
                    # bias-add on DVE: keeps the scalar engine out of the
                    # tail and overlaps with the attention recip/mul stream
                    nc.vector.tensor_scalar_add(
                        o4[:, mi * TB : (mi + 1) * TB],
                        ops_[:],
                        bp_sb[:, m : m + 1],
                    )
                nc.sync.dma_start(
                    y3[:, mg * 4 : (mg + 1) * 4, n * TB : (n + 1) * TB],
                    o4.rearrange("p (m t) -> p m t", t=TB),
                )

        q_block(0)
        q_block(1)
        q_block(2, halves=(0,))
        bds_all, nb_all = unpack_ar()
        a0 = pn_block(0, bds_all, nb_all)
        q_block(2, halves=(1,))
        a1 = pn_block(1, bds_all, nb_all)
        op_block(0, a0)
        q_block(3)
        a2 = pn_block(2, bds_all, nb_all)
        op_block(1, a1)
        a3 = pn_block(3, bds_all, nb_all)
        op_block(2, a2)
        op_block(3, a3)


_NC_CACHE = {}


def get_nc():
    if "nc" not in _NC_CACHE:
        _NC_CACHE["nc"] = build_nc()
    return _NC_CACHE["nc"]


def make_in_maps(x, Wqkv, Wg, bg, Wp, bp):
    bf = ml_dtypes.bfloat16
    x = np.asarray(x, dtype=np.float32)
    Wqkv = np.asarray(Wqkv, dtype=np.float32)
    Wg = np.asarray(Wg, dtype=np.float32)
    bg = np.asarray(bg, dtype=np.float32)
    Wp = np.asarray(Wp, dtype=np.float32)
    bp = np.asarray(bp, dtype=np.float32)

    wq = np.ascontiguousarray(Wqkv[:, :C]).astype(bf)
    wkv = np.ascontiguousarray(Wqkv[:, C:]).astype(bf)
    wgt = Wg.astype(bf)
    wp = Wp.astype(bf)
    bg2 = np.ascontiguousarray(bg.reshape(KC, 128).T)
    bp2 = np.ascontiguousarray(bp.reshape(KC, 128).T)

    xf = x.reshape(NCORES, T, C)
    in_maps = []
    for c in range(NCORES):
        xtc = np.ascontiguousarray(xf[c].T).astype(bf)
        in_maps.append(
            dict(
                xt=xtc, wq=wq, wkv=wkv, wgt=wgt, wp=wp,
                bg2=bg2, bp2=bp2,
            )
        )
    return in_maps


def kernel(x, Wqkv, Wg, bg, Wp, bp, _collect_perf=None):
    nc = get_nc()
    in_maps = make_in_maps(x, Wqkv, Wg, bg, Wp, bp)
    kwargs = {}
    if _collect_perf is not None:
        kwargs = dict(trace=True)
        if _collect_perf.get("tmpdir"):
            kwargs["tmpdir"] = _collect_perf["tmpdir"]
    res = run_bass_kernel_spmd(
        nc, in_maps, core_ids=list(range(NCORES)), **kwargs
    )
    if _collect_perf is not None:
        _collect_perf["exec_time_ns"] = res.exec_time_ns
        _collect_perf["results"] = res
    out = np.empty((NCORES, T, C), dtype=np.float32)
    for c in range(NCORES):
        out[c] = res.results[c]["y"].astype(np.float32).T
    return out.reshape(B, N, C)
